# revision 1
# baseline (speedup 1.0000x reference)
"""AttnBlock (GroupNorm + single-head self-attention + residual) on 8 TRN2 cores.

Sharding: data-parallel over batch B=8 -> one [64,64,128] image per core.
Per-core kernel layout notes:
  - xT/hT/qT/kT are [C=128 partitions, N=4096 free] (channels on partitions).
  - Scores are computed directly transposed: sT[k, q] = k_chunk @ qT so the
    probability matrix lands in [k-partition, q-free] layout, which is what
    the PV matmul needs (contraction over k on partitions).
  - Softmax skips max-subtraction (scores are O(1) here; exp can't overflow
    fp32) and the denominator Z is accumulated with an all-ones stationary
    matmul alongside PV. Z is then moved to q-partition layout with 8 tiny
    N=1 matmuls (lhsT = Z row-block, rhs = e0) so the normalization becomes
    a cheap per-partition scalar multiply in the epilogue.
  - The (q-block, k-chunk) loop is software-pipelined: score matmuls + exp
    run two steps ahead of the PV/Z accumulation so the PE never waits on
    the activation engine.
  - Matmuls run as float32r (fp32 bits, ~1 cycle/row on the PE at N=512).
"""

import sys

for _p in ("/opt/trn_rl_repo",):
    if _p not in sys.path:
        sys.path.insert(0, _p)

import numpy as np

import concourse.bass as bass
import concourse.tile as tile
from concourse import bacc, mybir
from concourse.bass_utils import run_bass_kernel_spmd
from concourse.tile import add_dep_helper

B, H, W, C = 8, 64, 64, 128
N = H * W  # 4096 positions per image
GROUPS = 32
GSIZE = C // GROUPS  # 4
EPS = 1e-6
NCORES = 8
P = 128
NT = N // P  # 32 position tiles / k-chunks
QB = 512  # q-block width of the attention main loop
NQB = N // QB  # 8
NSUB = QB // P  # 4 q-subtiles per block
SCALE = C ** -0.5

F32 = mybir.dt.float32
F32R = mybir.dt.float32r
BF16 = mybir.dt.bfloat16

# "f32r" | "f32" | "bf16" : dtype used by the big matmuls
MM_MODE = "f32r"


def _mm_store_dt():
    # dtype of tiles feeding the big matmuls; producers round on write
    # (the BIR verifier requires f32r matmul operands to be produced as f32r)
    return {"bf16": BF16, "f32r": F32R, "f32": F32}[MM_MODE]


def build_nc():
    nc = bacc.Bacc("TRN2", target_bir_lowering=False, debug=False)
    mdt = _mm_store_dt()

    x_d = nc.dram_tensor("x", [N, C], F32, kind="ExternalInput")
    xt_d = nc.dram_tensor("xt", [C, N], F32, kind="ExternalInput")
    wq_d = nc.dram_tensor("wq", [C, C], F32, kind="ExternalInput")
    wk_d = nc.dram_tensor("wk", [C, C], F32, kind="ExternalInput")
    wv_d = nc.dram_tensor("wv", [C, C], F32, kind="ExternalInput")
    wo_d = nc.dram_tensor("wo", [C, C], F32, kind="ExternalInput")
    bq_d = nc.dram_tensor("bq", [C], F32, kind="ExternalInput")
    bk_d = nc.dram_tensor("bk", [C], F32, kind="ExternalInput")
    bv_d = nc.dram_tensor("bv", [C], F32, kind="ExternalInput")
    bo_d = nc.dram_tensor("bo", [C], F32, kind="ExternalInput")
    gns_d = nc.dram_tensor("gn_scale", [C], F32, kind="ExternalInput")
    gnb_d = nc.dram_tensor("gn_bias", [C], F32, kind="ExternalInput")
    ident_d = nc.dram_tensor("ident", [P, P], F32, kind="ExternalInput")
    gmask_d = nc.dram_tensor("gmask", [P, P], F32, kind="ExternalInput")
    ones_d = nc.dram_tensor("onesm", [P, P], F32, kind="ExternalInput")
    out_d = nc.dram_tensor("out", [N, C], F32, kind="ExternalOutput")

    # DRAM views with positions split into [tile, partition]
    x_tiled = x_d.rearrange("(t p) c -> p t c", p=P)
    out_tiled = out_d.rearrange("(t p) c -> p t c", p=P)

    def col(ap_1d):
        # [C] dram -> [C, 1] partition-column view
        return ap_1d.unsqueeze(1)

    def brow(ap_1d):
        # [C] dram -> [128, C] partition-broadcast view (step-0 partition dim)
        return bass.AP(
            tensor=ap_1d.tensor, offset=ap_1d.offset, ap=[[0, P]] + list(ap_1d.ap)
        )

    with tile.TileContext(nc) as tc:
        with (
            tc.tile_pool(name="persist", bufs=1) as data,
            tc.tile_pool(name="small", bufs=1) as small,
            tc.tile_pool(name="onorm", bufs=NQB + 1) as onormpool,
        ):
            # ---- persistent SBUF tiles ----
            x_all = data.tile([P, NT, C], F32)  # x in [pos-in-tile, tile, C]
            xT = data.tile([P, N], F32)  # x transposed: [C, pos]
            hT = data.tile([P, N], mdt)  # groupnorm output, [C, pos]
            qT = data.tile([P, N], mdt)
            kT = data.tile([P, N], mdt)
            v_all = data.tile([P, NT, C], mdt)  # v in [pos-in-tile, tile, C]

            wq_s = small.tile([C, C], mdt)
            wk_s = small.tile([C, C], mdt)
            wv_s = small.tile([C, C], mdt)
            wo_s = small.tile([C, C], mdt)
            ident_s = small.tile([P, P], F32)
            gmask_s = small.tile([P, P], F32)
            ones_s = small.tile([P, P], mdt)
            bq_s = small.tile([C, 1], F32)
            bk_s = small.tile([C, 1], F32)
            bv_r = small.tile([P, C], F32)  # bv broadcast to all partitions
            bo_r = small.tile([P, C], F32)
            gns_s = small.tile([C, 1], F32)
            gnb_s = small.tile([C, 1], F32)
            eps_s = small.tile([C, 1], F32)

            # xT (host-pretransposed x) gates everything: split across both
            # DMA queues so chunks land ASAP; x_all (residual, needed late)
            # and the small constants follow.
            XCH = 4
            for ci in range(XCH):
                cs = slice(ci * N // XCH, (ci + 1) * N // XCH)
                eng = nc.sync if ci % 2 == 0 else nc.gpsimd
                eng.dma_start(xT[:, cs], xt_d[:, cs])
            # GN-chain constants next on the sync queue (the gpsimd queue
            # is busy with weights; the GN chain must not wait on them)
            nc.sync.dma_start(ident_s[:], ident_d[:])
            nc.sync.dma_start(gmask_s[:], gmask_d[:])
            nc.sync.dma_start(gns_s[:], col(gns_d[:]))
            nc.sync.dma_start(gnb_s[:], col(gnb_d[:]))
            nc.sync.dma_start(bq_s[:], col(bq_d[:]))
            nc.sync.dma_start(bk_s[:], col(bk_d[:]))
            for ci in range(XCH):
                ts = slice(ci * NT // XCH, (ci + 1) * NT // XCH)
                eng = nc.sync if ci < 2 else nc.gpsimd
                eng.dma_start(x_all[:, ts, :], x_tiled[:, ts, :])

            def ld2(dst, src):
                # SWDGE load; for f32r stage as fp32 then round on DVE (the
                # BIR verifier requires f32r matmul operands produced as f32r)
                if MM_MODE == "f32r":
                    stg = small.tile(list(dst.shape), F32, tag="wstage")
                    nc.gpsimd.dma_start(stg[:], src)
                    nc.vector.tensor_copy(dst[:], stg[:])
                else:
                    nc.gpsimd.dma_start(dst[:], src)

            ld2(wq_s[:], wq_d[:])
            ld2(wk_s[:], wk_d[:])
            ld2(wv_s[:], wv_d[:])
            ld2(wo_s[:], wo_d[:])
            ld2(ones_s[:], ones_d[:])
            nc.gpsimd.dma_start(bv_r[:], brow(bv_d[:]))
            nc.gpsimd.dma_start(bo_r[:], brow(bo_d[:]))
            nc.vector.memset(eps_s[:], EPS)

            # ---- phase 1+2: group norm stats straight off the xT DMA ----
            stats = small.tile([P, 16, nc.vector.BN_STATS_DIM], F32)
            with tc.tile_pool(name="tp", bufs=3, space="PSUM") as tpsum:
                for j in range(16):
                    nc.vector.bn_stats(
                        out=stats[:, j, :], in_=xT[:, j * 256 : (j + 1) * 256]
                    )
                    # keep the PE's HAM activity monitor busy through the
                    # DVE-bound stats/GN window so the projections start at
                    # full clock (idle >3.4us re-throttles the array). The
                    # stats-slice input paces these with the DVE stream.
                    pt = tpsum.tile([P, P], F32, tag="tp")
                    nc.tensor.transpose(
                        pt[0:6, :], stats[:, j, :], ident_s[:]
                    )
                mv = small.tile([P, nc.vector.BN_AGGR_DIM], F32)
                nc.vector.bn_aggr(out=mv[:], in_=stats[:])
                # per-channel [mean, E[x^2]] -> group-averaged via mask matmul
                st2 = small.tile([P, 2], F32)
                nc.vector.tensor_copy(st2[:, 0:1], mv[:, 0:1])
                msq = small.tile([P, 1], F32)
                nc.vector.tensor_mul(msq[:], mv[:, 0:1], mv[:, 0:1])
                nc.vector.tensor_add(st2[:, 1:2], mv[:, 1:2], msq[:])
                gpsum = tpsum.tile([P, 2], F32, tag="tp")
                nc.tensor.matmul(gpsum[:], gmask_s[:], st2[:])
                gstat = small.tile([P, 2], F32)
                nc.vector.tensor_copy(gstat[:], gpsum[:])

                # var_g = E_g[x^2] - mean_g^2 ; rstd = 1/sqrt(var_g + eps)
                varg = small.tile([P, 1], F32)
                nc.vector.tensor_mul(varg[:], gstat[:, 0:1], gstat[:, 0:1])
                nc.vector.tensor_tensor(
                    varg[:], gstat[:, 1:2], varg[:], mybir.AluOpType.subtract
                )
                nc.scalar.activation(
                    out=varg[:],
                    in_=varg[:],
                    func=mybir.ActivationFunctionType.Sqrt,
                    bias=eps_s[:],
                    scale=1.0,
                )
                rstd = small.tile([P, 1], F32)
                nc.vector.reciprocal(rstd[:], varg[:])
                # h = x * A + Bc with A = rstd*scale, Bc = bias - mean*A
                A_s = small.tile([P, 1], F32)
                nc.vector.tensor_mul(A_s[:], rstd[:], gns_s[:])
                mA = small.tile([P, 1], F32)
                nc.vector.tensor_mul(mA[:], gstat[:, 0:1], A_s[:])
                Bc_s = small.tile([P, 1], F32)
                nc.vector.tensor_tensor(
                    Bc_s[:], gnb_s[:], mA[:], mybir.AluOpType.subtract
                )
                # hT in 8 chunks so projections can start early; alternate
                # ACT (Identity(in*scale + bias)) and DVE to split the work
                for j in range(8):
                    sl = slice(j * 512, (j + 1) * 512)
                    if j % 2 == 0:
                        nc.scalar.activation(
                            out=hT[:, sl],
                            in_=xT[:, sl],
                            func=mybir.ActivationFunctionType.Identity,
                            scale=A_s[:],
                            bias=Bc_s[:],
                        )
                    else:
                        nc.vector.tensor_scalar(
                            out=hT[:, sl],
                            in0=xT[:, sl],
                            scalar1=A_s[:],
                            scalar2=Bc_s[:],
                            op0=mybir.AluOpType.mult,
                            op1=mybir.AluOpType.add,
                        )

            # ---- phase 3: projections qT/kT [C,N], v [pos,C] ----
            with (
                tc.tile_pool(name="pq", bufs=3, space="PSUM") as pqpool,
                tc.tile_pool(name="pv", bufs=3, space="PSUM") as pvpool,
            ):
                # emission order favors what the attention loop needs first:
                # qT block 0 (j=0,1), all of kT, all of v, then the rest of qT
                def emit_q(j):
                    sl = slice(j * 512, (j + 1) * 512)
                    pq = pqpool.tile([P, 512], F32, tag="pq")
                    nc.tensor.matmul(pq[:], wq_s[:], hT[:, sl])
                    nc.scalar.activation(
                        out=qT[:, sl],
                        in_=pq[:],
                        func=mybir.ActivationFunctionType.Identity,
                        bias=bq_s[:],
                    )

                for j in range(2):
                    emit_q(j)
                for j in range(8):
                    sl = slice(j * 512, (j + 1) * 512)
                    pk = pqpool.tile([P, 512], F32, tag="pq")
                    nc.tensor.matmul(pk[:], wk_s[:], hT[:, sl])
                    nc.vector.tensor_scalar_add(kT[:, sl], pk[:], bk_s[:])
                # v directly in [pos, C] layout (hT slice stationary)
                for i in range(NT):
                    pv = pvpool.tile([P, C], F32, tag="pv")
                    nc.tensor.matmul(pv[:], hT[:, i * P : (i + 1) * P], wv_s[:])
                    nc.any.tensor_add(v_all[:, i, :], pv[:], bv_r[:])
                for j in range(2, 8):
                    emit_q(j)

            # ---- phase 4: attention, software-pipelined over 256 steps ----
            # step = (qb, kc): scores for two consecutive kc share one PSUM
            # pair-tile and one exp; oT/Z accumulators are double-buffered
            # (1 bank each at QB=512) so block boundaries don't stall the PE.
            with (
                tc.tile_pool(name="sT", bufs=2, space="PSUM") as sTpool,
                tc.tile_pool(name="oT", bufs=2, space="PSUM") as oTpool,
                tc.tile_pool(name="Zp", bufs=2, space="PSUM") as zpool,
                tc.tile_pool(name="pexp", bufs=4) as pexppool,
            ):
                NSTEP = NQB * NT  # 256
                sT_pairs = {}
                pexp_tiles = {}
                psum_oT = {}
                psum_Z = {}
                tail_state = {}

                def emit_scores(step):
                    qb, kc = divmod(step, NT)
                    q0 = qb * QB
                    ksl = slice(kc * P, (kc + 1) * P)
                    half = kc % 2
                    if half == 0:
                        sT_pairs[step // 2] = sTpool.tile(
                            [P, 2, QB], F32, tag="sT", name=f"sT{step}"
                        )
                    psum_sT = sT_pairs[step // 2]
                    nc.tensor.matmul(
                        psum_sT[:, half, :],
                        kT[:, ksl],
                        qT[:, q0 : q0 + QB],
                    )
                    if half == 1:
                        pexp = pexppool.tile(
                            [P, 2, QB], _mm_store_dt(), tag="pexp"
                        )
                        nc.scalar.activation(
                            out=pexp[:],
                            in_=psum_sT[:],
                            func=mybir.ActivationFunctionType.Exp,
                            scale=SCALE,
                        )
                        pexp_tiles[step // 2] = pexp

                def emit_pvz(step):
                    qb, kc = divmod(step, NT)
                    if kc == 0:
                        psum_oT[qb] = oTpool.tile(
                            [P, QB], F32, tag="oT", name=f"psum_oT_{qb}"
                        )
                        psum_Z[qb] = zpool.tile(
                            [P, QB], F32, tag="Z", name=f"psum_Z_{qb}"
                        )
                    pexp = pexp_tiles[step // 2]
                    if kc % 2 == 1:
                        del pexp_tiles[step // 2]
                    first, last = kc == 0, kc == NT - 1
                    nc.tensor.matmul(
                        psum_oT[qb][:],
                        v_all[:, kc, :],
                        pexp[:, kc % 2, :],
                        start=first,
                        stop=last,
                    )
                    nc.tensor.matmul(
                        psum_Z[qb][:],
                        ones_s[:],
                        pexp[:, kc % 2, :],
                        start=first,
                        stop=last,
                    )

                def emit_tail_head(qb):
                    """Evacuate oT/Z PSUM -> SBUF; Z to q-partition layout
                    via tiny N=1 matmuls into a view of the freed Z psum
                    tile; 1/Z on DVE."""
                    poT, pZ = psum_oT.pop(qb), psum_Z.pop(qb)
                    oT_sb = onormpool.tile(
                        [P, QB], _mm_store_dt(), tag="on", name=f"oTsb{qb}"
                    )
                    ci = nc.vector.tensor_copy(oT_sb[:], poT[:])
                    # x_all += bo for this block, pinned behind the oT copy
                    # so the scheduler can't float it into the startup-
                    # critical GN window (it has no natural early deps)
                    xsl = x_all[:, qb * NSUB : (qb + 1) * NSUB, :]
                    bi = nc.vector.tensor_add(
                        xsl, xsl, bo_r[:, None, :].to_broadcast((P, NSUB, C))
                    )
                    add_dep_helper(
                        bi.ins, ci.ins, sync=False, reason="defer bo-add"
                    )
                    Z_sb = onormpool.tile([P, QB], F32, tag="zsb", name=f"Zsb{qb}")
                    nc.vector.tensor_copy(Z_sb[:], pZ[:])
                    zq_psum = pZ[:, 0:NSUB]
                    for s in range(NSUB):
                        nc.tensor.matmul(
                            zq_psum[:, s : s + 1],
                            Z_sb[:, s * P : (s + 1) * P],
                            ident_s[:, 0:1],
                        )
                    rzq = onormpool.tile([P, NSUB], F32, tag="rzq", name=f"rzq{qb}")
                    nc.vector.reciprocal(rzq[:], zq_psum)
                    ostage = onormpool.tile(
                        [P, NSUB, C], F32, tag="os", name=f"ost{qb}"
                    )
                    tail_state[qb] = (oT_sb, rzq, ostage)

                def emit_outproj(qb, s, pool, tag):
                    """One q-subtile of a finished block's out-projection."""
                    oT_sb, rzq, ostage = tail_state[qb]
                    pop = pool.tile([P, C], F32, tag=tag, name=f"po{qb}_{s}")
                    nc.tensor.matmul(pop[:], oT_sb[:, s * P : (s + 1) * P], wo_s[:])
                    # out = attn/Z + (x + bo)
                    nc.vector.scalar_tensor_tensor(
                        out=ostage[:, s, :],
                        in0=pop[:],
                        scalar=rzq[:, s : s + 1],
                        in1=x_all[:, qb * NSUB + s, :],
                        op0=mybir.AluOpType.mult,
                        op1=mybir.AluOpType.add,
                    )
                    if s == NSUB - 1:
                        del tail_state[qb]
                        nc.sync.dma_start(
                            out_tiled[:, qb * NSUB : (qb + 1) * NSUB, :],
                            ostage[:],
                        )

                LOOKAHEAD = 4
                for step in range(LOOKAHEAD):
                    emit_scores(step)
                for step in range(NSTEP):
                    qb, kc = divmod(step, NT)
                    emit_pvz(step)
                    if step + LOOKAHEAD < NSTEP:
                        emit_scores(step + LOOKAHEAD)
                    if kc == NT - 1:
                        emit_tail_head(qb)
                    if qb == NQB - 1 and kc < (NQB - 1) * NSUB:
                        # blocks 0..6 out-project in the idle oT psum slot
                        # while the last block's attention still runs
                        emit_outproj(kc // NSUB, kc % NSUB, oTpool, "oT")

                # last block's own out-projection (its accumulator just
                # finished; reuse the same slot rotation)
                for s in range(NSUB):
                    emit_outproj(NQB - 1, s, oTpool, "oT")

    nc.compile()
    return nc


_NC_CACHE = {}


def _get_nc():
    key = MM_MODE
    if key not in _NC_CACHE:
        _NC_CACHE[key] = build_nc()
    return _NC_CACHE[key]


def make_in_maps(**inputs):
    x = np.ascontiguousarray(np.asarray(inputs["x"], dtype=np.float32))
    ident = np.eye(P, dtype=np.float32)
    gmask = (
        np.kron(np.eye(GROUPS, dtype=np.float32), np.ones((GSIZE, GSIZE), np.float32))
        / GSIZE
    )
    onesm = np.ones((P, P), dtype=np.float32)
    shared = {
        "wq": np.asarray(inputs["wq"], np.float32),
        "wk": np.asarray(inputs["wk"], np.float32),
        "wv": np.asarray(inputs["wv"], np.float32),
        "wo": np.asarray(inputs["wo"], np.float32),
        "bq": np.asarray(inputs["bq"], np.float32),
        "bk": np.asarray(inputs["bk"], np.float32),
        "bv": np.asarray(inputs["bv"], np.float32),
        "bo": np.asarray(inputs["bo"], np.float32),
        "gn_scale": np.asarray(inputs["gn_scale"], np.float32),
        "gn_bias": np.asarray(inputs["gn_bias"], np.float32),
        "ident": ident,
        "gmask": gmask,
        "onesm": onesm,
    }
    return [
        {
            "x": x[b].reshape(N, C),
            "xt": np.ascontiguousarray(x[b].reshape(N, C).T),
            **shared,
        }
        for b in range(B)
    ]


def kernel(**inputs):
    nc = _get_nc()
    in_maps = make_in_maps(**inputs)
    res = run_bass_kernel_spmd(nc, in_maps, core_ids=list(range(NCORES)))
    out = np.stack([res.results[b]["out"] for b in range(B)], axis=0)
    return out.reshape(B, H, W, C).astype(np.float32)


if __name__ == "__main__":
    rng = np.random.default_rng(0)
    ins = {
        "x": rng.standard_normal((B, H, W, C), dtype=np.float32),
        "gn_scale": np.ones(C, np.float32),
        "gn_bias": np.zeros(C, np.float32),
    }
    for w in ("wq", "wk", "wv", "wo"):
        ins[w] = rng.standard_normal((C, C), dtype=np.float32) * SCALE
    for b in ("bq", "bk", "bv", "bo"):
        ins[b] = np.zeros(C, np.float32)
    o = kernel(**ins)
    print("out", o.shape, o.dtype, float(np.abs(o).max()))



# revision 5
# speedup vs baseline: 1.2909x; 1.2909x over previous
"""AttnBlock (GroupNorm + single-head self-attention + residual) on 8 TRN2 cores.

Sharding: data-parallel over batch B=8 -> one [64,64,128] image per core.

Per-core kernel design (v2, fp8/bf16):
  - xT/hT/qT/kT are [C=128 partitions, N=4096 free] (channels on partitions).
  - Projections and score matmuls run in bf16 (1 cyc/row like f32r, but the
    128-col LDWEIGHTS uses fast-weight-load and overlaps the matmul stream,
    unlike f32r whose weight load serializes with the matmul).
  - Scores land transposed sT[k, q] = kT_chunk.T @ qT so the probability
    matrix is in [k-partition, q-free] layout for the PV contraction.
  - qT is pre-scaled by A_Q = 8*log2e/sqrt(C) so the score PSUM is directly
    the Schraudolph exponent: pexp bits = u8(sat(psum + B_SCH)) reinterpreted
    as fp8e4m3 gives exp(score - M_SHIFT) to ~3% (DVE path, one tensor_scalar
    with op0=add, op1=max-0 for the underflow clamp). The ACT path computes
    the exact exp via activation(Exp, scale=ln2/8, bias=-M_SHIFT) into fp8.
    Splitting the 16.7M exps between both engines keeps softmax off the
    critical path. M_SHIFT=4 centers exp(s-4) in e4m3 range (max observed
    score ~8.3, fp8 overflow at 10.05).
  - pexp tiles are [128, 2, 512] fp8 pairs; PV uses fp8 DoubleRow matmuls
    (0.5 cyc/row): one matmul per k-chunk pair with v pairs [128, 2, 128].
  - The softmax denominator Z accumulates via all-ones DoubleRow matmuls,
    grouped per q-block (16 back-to-back MMs share one LDWEIGHTS) while the
    next block's scores start.
  - The out-projection is transposed: stationary wo, moving (oT/Z) -> output
    in [C, q] layout, so the residual add uses xT directly (x_all and its
    2MB DMA are gone) and the epilogue is one scalar_tensor_tensor:
    out = (pop + bo2) + xT, with bo2 = bo + bv@wo folded host-side.
    Output DMA writes a transposed [C, N] dram tensor; host transposes back.
"""

import sys

for _p in ("/opt/trn_rl_repo",):
    if _p not in sys.path:
        sys.path.insert(0, _p)

import numpy as np

import concourse.bass as bass
import concourse.tile as tile
from concourse import bacc, mybir
from concourse.bass_utils import run_bass_kernel_spmd

B, H, W, C = 8, 64, 64, 128
N = H * W  # 4096 positions per image
GROUPS = 32
GSIZE = C // GROUPS  # 4
EPS = 1e-6
NCORES = 8
P = 128
NT = N // P  # 32 k-chunks
QB = 512  # q-block width
NQB = N // QB  # 8
NPAIR = NT // 2  # 16 k-chunk pairs per q-block
SCALE = C ** -0.5
LOG2E = 1.4426950408889634
M_SHIFT = 4.0  # softmax shift: pexp = exp(s - M_SHIFT)
A_Q = 8.0 * LOG2E * SCALE  # baked into qT so score psum = schraudolph exponent
B_SCH = 8.0 * (7.0 - LOG2E * M_SHIFT) + 0.5  # +0.5 compensates trunc-on-convert
ACT_SCALE = 1.0 / (8.0 * LOG2E)  # un-bake A_Q: exp(psum*ACT_SCALE - M_SHIFT)
ESPLIT = 640  # of each 1024-elem pair tile: ACT does [0:640], DVE the rest

F32 = mybir.dt.float32
BF16 = mybir.dt.bfloat16
F8 = mybir.dt.float8e4
U8 = mybir.dt.uint8
DR = mybir.MatmulPerfMode.DoubleRow


def build_nc():
    nc = bacc.Bacc("TRN2", target_bir_lowering=False, debug=False)

    xt_d = nc.dram_tensor("xt", [C, N], F32, kind="ExternalInput")
    wq_d = nc.dram_tensor("wq", [C, C], BF16, kind="ExternalInput")
    wk_d = nc.dram_tensor("wk", [C, C], BF16, kind="ExternalInput")
    wv_d = nc.dram_tensor("wv", [C, C], BF16, kind="ExternalInput")
    wo_d = nc.dram_tensor("wo", [C, C], BF16, kind="ExternalInput")
    bqs_d = nc.dram_tensor("bqs", [C], F32, kind="ExternalInput")  # bq * A_Q
    bk_d = nc.dram_tensor("bk", [C], F32, kind="ExternalInput")
    bo2_d = nc.dram_tensor("bo2", [C], F32, kind="ExternalInput")  # bo + bv@wo
    gns_d = nc.dram_tensor("gn_scale", [C], F32, kind="ExternalInput")
    gnb_d = nc.dram_tensor("gn_bias", [C], F32, kind="ExternalInput")
    ident_d = nc.dram_tensor("ident", [P, P], F32, kind="ExternalInput")
    gmask_d = nc.dram_tensor("gmask", [P, P], F32, kind="ExternalInput")
    out_d = nc.dram_tensor("outT", [C, N], F32, kind="ExternalOutput")

    def col(ap_1d):
        # [C] dram -> [C, 1] partition-column view
        return ap_1d.unsqueeze(1)

    with tile.TileContext(nc) as tc:
        with (
            tc.tile_pool(name="persist", bufs=1) as data,
            tc.tile_pool(name="small", bufs=1) as small,
            tc.tile_pool(name="pexp", bufs=NPAIR + 3) as pexppool,
            tc.tile_pool(name="epi", bufs=3) as epipool,
        ):
            # ---- persistent SBUF tiles ----
            xT = data.tile([P, N], F32)
            hT = data.tile([P, N], BF16)
            qTs = data.tile([P, N], BF16)  # q, pre-scaled by A_Q
            kT = data.tile([P, N], BF16)
            v_all = data.tile([P, NT, C], F8)

            wq_s = small.tile([C, C], BF16)
            wk_s = small.tile([C, C], BF16)
            wv_s = small.tile([C, C], BF16)
            wo_s = small.tile([C, C], BF16)
            ident_s = small.tile([P, P], F32)
            gmask_s = small.tile([P, P], F32)
            ones2 = small.tile([P, 2, C], F8)
            bqs_s = small.tile([C, 1], F32)
            bk_s = small.tile([C, 1], F32)
            bo2_s = small.tile([C, 1], F32)
            gns_s = small.tile([C, 1], F32)
            gnb_s = small.tile([C, 1], F32)
            eps_s = small.tile([C, 1], F32)
            negm_s = small.tile([C, 1], F32)

            # xT gates everything: split across both DMA queues.
            XCH = 4
            for ci in range(XCH):
                cs = slice(ci * N // XCH, (ci + 1) * N // XCH)
                eng = nc.sync if ci % 2 == 0 else nc.gpsimd
                eng.dma_start(xT[:, cs], xt_d[:, cs])
            # GN-chain constants on the sync queue, weights on gpsimd.
            nc.sync.dma_start(ident_s[:], ident_d[:])
            nc.sync.dma_start(gmask_s[:], gmask_d[:])
            nc.sync.dma_start(gns_s[:], col(gns_d[:]))
            nc.sync.dma_start(gnb_s[:], col(gnb_d[:]))
            nc.sync.dma_start(bqs_s[:], col(bqs_d[:]))
            nc.sync.dma_start(bk_s[:], col(bk_d[:]))
            nc.sync.dma_start(bo2_s[:], col(bo2_d[:]))
            nc.gpsimd.dma_start(wq_s[:], wq_d[:])
            nc.gpsimd.dma_start(wk_s[:], wk_d[:])
            nc.gpsimd.dma_start(wv_s[:], wv_d[:])
            nc.gpsimd.dma_start(wo_s[:], wo_d[:])
            nc.gpsimd.memset(ones2[:], 1.0)
            nc.vector.memset(eps_s[:], EPS)
            nc.vector.memset(negm_s[:], -M_SHIFT)

            # ---- phase 1+2: group norm stats straight off the xT DMA ----
            stats = small.tile([P, 16, nc.vector.BN_STATS_DIM], F32)
            with tc.tile_pool(name="tp", bufs=3, space="PSUM") as tpsum:
                for j in range(16):
                    nc.vector.bn_stats(
                        out=stats[:, j, :], in_=xT[:, j * 256 : (j + 1) * 256]
                    )
                    # keep the PE's HAM activity monitor busy through the
                    # DVE-bound stats/GN window so the attention matmuls
                    # start at full clock (idle >3.4us re-throttles).
                    pt = tpsum.tile([P, P], F32, tag="tp")
                    nc.tensor.transpose(
                        pt[0:6, :], stats[:, j, :], ident_s[:]
                    )
                mv = small.tile([P, nc.vector.BN_AGGR_DIM], F32)
                nc.vector.bn_aggr(out=mv[:], in_=stats[:])
                # per-channel [mean, E[x^2]] -> group-averaged via mask matmul
                st2 = small.tile([P, 2], F32)
                nc.vector.tensor_copy(st2[:, 0:1], mv[:, 0:1])
                msq = small.tile([P, 1], F32)
                nc.vector.tensor_mul(msq[:], mv[:, 0:1], mv[:, 0:1])
                nc.vector.tensor_add(st2[:, 1:2], mv[:, 1:2], msq[:])
                gpsum = tpsum.tile([P, 2], F32, tag="tp")
                nc.tensor.matmul(gpsum[:], gmask_s[:], st2[:])
                gstat = small.tile([P, 2], F32)
                nc.vector.tensor_copy(gstat[:], gpsum[:])

                # var_g = E_g[x^2] - mean_g^2 ; rstd = 1/sqrt(var_g + eps)
                varg = small.tile([P, 1], F32)
                nc.vector.tensor_mul(varg[:], gstat[:, 0:1], gstat[:, 0:1])
                nc.vector.tensor_tensor(
                    varg[:], gstat[:, 1:2], varg[:], mybir.AluOpType.subtract
                )
                nc.scalar.activation(
                    out=varg[:],
                    in_=varg[:],
                    func=mybir.ActivationFunctionType.Sqrt,
                    bias=eps_s[:],
                    scale=1.0,
                )
                rstd = small.tile([P, 1], F32)
                nc.vector.reciprocal(rstd[:], varg[:])
                # h = x * A + Bc with A = rstd*scale, Bc = bias - mean*A
                A_s = small.tile([P, 1], F32)
                nc.vector.tensor_mul(A_s[:], rstd[:], gns_s[:])
                mA = small.tile([P, 1], F32)
                nc.vector.tensor_mul(mA[:], gstat[:, 0:1], A_s[:])
                Bc_s = small.tile([P, 1], F32)
                nc.vector.tensor_tensor(
                    Bc_s[:], gnb_s[:], mA[:], mybir.AluOpType.subtract
                )
                # hT (bf16) in 8 chunks; alternate ACT and DVE
                for j in range(8):
                    sl = slice(j * 512, (j + 1) * 512)
                    if j % 2 == 0:
                        nc.scalar.activation(
                            out=hT[:, sl],
                            in_=xT[:, sl],
                            func=mybir.ActivationFunctionType.Identity,
                            scale=A_s[:],
                            bias=Bc_s[:],
                        )
                    else:
                        nc.vector.tensor_scalar(
                            out=hT[:, sl],
                            in0=xT[:, sl],
                            scalar1=A_s[:],
                            scalar2=Bc_s[:],
                            op0=mybir.AluOpType.mult,
                            op1=mybir.AluOpType.add,
                        )

            # ---- phase 3: projections qTs/kT [C,N] bf16, v [pos,C] fp8 ----
            with (
                tc.tile_pool(name="pq", bufs=3, space="PSUM") as pqpool,
                tc.tile_pool(name="pv", bufs=3, space="PSUM") as pvpool,
            ):
                def emit_q(j):
                    sl = slice(j * 512, (j + 1) * 512)
                    pq = pqpool.tile([P, 512], F32, tag="pq")
                    nc.tensor.matmul(pq[:], wq_s[:], hT[:, sl])
                    # qTs = A_Q*(h@wq) + A_Q*bq  (score psum = schraudolph t)
                    nc.scalar.activation(
                        out=qTs[:, sl],
                        in_=pq[:],
                        func=mybir.ActivationFunctionType.Identity,
                        scale=A_Q,
                        bias=bqs_s[:],
                    )

                for j in range(2):
                    emit_q(j)
                for j in range(8):
                    sl = slice(j * 512, (j + 1) * 512)
                    pk = pqpool.tile([P, 512], F32, tag="pq")
                    nc.tensor.matmul(pk[:], wk_s[:], hT[:, sl])
                    nc.vector.tensor_scalar_add(kT[:, sl], pk[:], bk_s[:])
                # v in [pos, C] fp8 (hT slice stationary); bias folded to bo2
                for i in range(NT):
                    pv = pvpool.tile([P, C], F32, tag="pv")
                    nc.tensor.matmul(pv[:], hT[:, i * P : (i + 1) * P], wv_s[:])
                    nc.scalar.copy(v_all[:, i, :], pv[:])
                for j in range(2, 8):
                    emit_q(j)

            # ---- phase 4: attention over (q-block, k-chunk-pair) steps ----
            with (
                tc.tile_pool(name="sT", bufs=2, space="PSUM") as sTpool,
                tc.tile_pool(name="oT", bufs=2, space="PSUM") as oTpool,
                tc.tile_pool(name="Zp", bufs=2, space="PSUM") as zpool,
            ):
                NSTEP = NQB * NPAIR  # 128 pair-steps
                sT_psums = {}
                pexp_tiles = {}
                psum_oT = {}
                psum_Z = {}

                def emit_scores(p):
                    qb, j = divmod(p, NPAIR)
                    q0 = qb * QB
                    psum_sT = sTpool.tile([P, 2, QB], F32, tag="sT", name=f"sT{p}")
                    sT_psums[p] = psum_sT
                    for h in range(2):
                        kc = 2 * j + h
                        nc.tensor.matmul(
                            psum_sT[:, h, :],
                            kT[:, kc * P : (kc + 1) * P],
                            qTs[:, q0 : q0 + QB],
                        )
                    pexp = pexppool.tile([P, 2, QB], F8, tag="pexp", name=f"pe{p}")
                    pexp_tiles[p] = pexp
                    pe_flat = pexp[:].rearrange("p a b -> p (a b)")
                    ps_flat = psum_sT[:].rearrange("p a b -> p (a b)")
                    # ACT: exact exp(s - M) into fp8
                    nc.scalar.activation(
                        out=pe_flat[:, 0:ESPLIT],
                        in_=ps_flat[:, 0:ESPLIT],
                        func=mybir.ActivationFunctionType.Exp,
                        scale=ACT_SCALE,
                        bias=negm_s[:],
                    )
                    # DVE: schraudolph fp8 bits = sat_u8(max(t + B_SCH, 0))
                    nc.vector.tensor_scalar(
                        out=pe_flat[:, ESPLIT:].bitcast(U8),
                        in0=ps_flat[:, ESPLIT:],
                        scalar1=B_SCH,
                        scalar2=0.0,
                        op0=mybir.AluOpType.add,
                        op1=mybir.AluOpType.max,
                    )

                def emit_pv(p):
                    qb, j = divmod(p, NPAIR)
                    if j == 0:
                        psum_oT[qb] = oTpool.tile(
                            [P, QB], F32, tag="oT", name=f"oT{qb}"
                        )
                    nc.tensor.matmul(
                        psum_oT[qb][:],
                        v_all[:, 2 * j : 2 * j + 2, :],
                        pexp_tiles[p][:],
                        start=(j == 0),
                        stop=(j == NPAIR - 1),
                        perf_mode=DR,
                    )
                    del sT_psums[p]

                def emit_z_group(qb):
                    psum_Z[qb] = zpool.tile([P, QB], F32, tag="Z", name=f"Z{qb}")
                    for j in range(NPAIR):
                        nc.tensor.matmul(
                            psum_Z[qb][:],
                            ones2[:],
                            pexp_tiles[qb * NPAIR + j][:],
                            start=(j == 0),
                            stop=(j == NPAIR - 1),
                            perf_mode=DR,
                        )
                    for j in range(NPAIR):
                        del pexp_tiles[qb * NPAIR + j]

                def emit_epilogue(qb):
                    qsl = slice(qb * QB, (qb + 1) * QB)
                    poT, pZ = psum_oT.pop(qb), psum_Z.pop(qb)
                    rZ = epipool.tile([P, QB], F32, tag="rZ", name=f"rZ{qb}")
                    nc.vector.reciprocal(rZ[:], pZ[:])
                    oTn = epipool.tile([P, QB], BF16, tag="oTn", name=f"oTn{qb}")
                    nc.vector.tensor_mul(oTn[:], poT[:], rZ[:])
                    pop = zpool.tile([P, QB], F32, tag="Z", name=f"pop{qb}")
                    nc.tensor.matmul(pop[:], wo_s[:], oTn[:])
                    outsb = epipool.tile([P, QB], F32, tag="ob", name=f"ob{qb}")
                    # out = (pop + bo2) + xT   (residual + folded biases)
                    nc.vector.scalar_tensor_tensor(
                        out=outsb[:],
                        in0=pop[:],
                        scalar=bo2_s[:],
                        in1=xT[:, qsl],
                        op0=mybir.AluOpType.add,
                        op1=mybir.AluOpType.add,
                    )
                    nc.sync.dma_start(out_d[:, qsl], outsb[:])

                LA = 2  # pair-steps of score/exp lookahead ahead of PV
                for p in range(LA):
                    emit_scores(p)
                for p in range(NSTEP):
                    qb, j = divmod(p, NPAIR)
                    emit_pv(p)
                    if p + LA < NSTEP:
                        emit_scores(p + LA)
                    if j == NPAIR - 1:
                        emit_z_group(qb)
                        emit_epilogue(qb)

    nc.compile()
    return nc


_NC_CACHE = {}


def _get_nc():
    if "nc" not in _NC_CACHE:
        _NC_CACHE["nc"] = build_nc()
    return _NC_CACHE["nc"]


def make_in_maps(**inputs):
    bf16 = mybir.dt.np(BF16)
    x = np.ascontiguousarray(np.asarray(inputs["x"], dtype=np.float32))
    ident = np.eye(P, dtype=np.float32)
    gmask = (
        np.kron(np.eye(GROUPS, dtype=np.float32), np.ones((GSIZE, GSIZE), np.float32))
        / GSIZE
    )
    wo64 = np.asarray(inputs["wo"], np.float64)
    bo2 = (
        np.asarray(inputs["bo"], np.float64)
        + np.asarray(inputs["bv"], np.float64) @ wo64
    ).astype(np.float32)
    bqs = (np.asarray(inputs["bq"], np.float64) * A_Q).astype(np.float32)
    shared = {
        "wq": np.asarray(inputs["wq"], np.float32).astype(bf16),
        "wk": np.asarray(inputs["wk"], np.float32).astype(bf16),
        "wv": np.asarray(inputs["wv"], np.float32).astype(bf16),
        "wo": np.asarray(inputs["wo"], np.float32).astype(bf16),
        "bqs": bqs,
        "bk": np.asarray(inputs["bk"], np.float32),
        "bo2": bo2,
        "gn_scale": np.asarray(inputs["gn_scale"], np.float32),
        "gn_bias": np.asarray(inputs["gn_bias"], np.float32),
        "ident": ident,
        "gmask": gmask,
    }
    return [
        {
            "xt": np.ascontiguousarray(x[b].reshape(N, C).T),
            **shared,
        }
        for b in range(B)
    ]


def kernel(**inputs):
    nc = _get_nc()
    in_maps = make_in_maps(**inputs)
    res = run_bass_kernel_spmd(nc, in_maps, core_ids=list(range(NCORES)))
    out = np.stack(
        [np.asarray(res.results[b]["outT"]).T for b in range(B)], axis=0
    )
    return out.reshape(B, H, W, C).astype(np.float32)


if __name__ == "__main__":
    rng = np.random.default_rng(0)
    ins = {
        "x": rng.standard_normal((B, H, W, C), dtype=np.float32),
        "gn_scale": np.ones(C, np.float32),
        "gn_bias": np.zeros(C, np.float32),
    }
    for w in ("wq", "wk", "wv", "wo"):
        ins[w] = rng.standard_normal((C, C), dtype=np.float32) * SCALE
    for b in ("bq", "bk", "bv", "bo"):
        ins[b] = np.zeros(C, np.float32)
    o = kernel(**ins)
    print("out", o.shape, o.dtype, float(np.abs(o).max()))


# revision 12
# speedup vs baseline: 1.5555x; 1.2050x over previous
"""AttnBlock (GroupNorm + single-head self-attention + residual) on 8 TRN2 cores.

Sharding: data-parallel over batch B=8 -> one [64,64,128] image per core.

Per-core kernel design (v2, fp8/bf16):
  - xT/hT/qT/kT are [C=128 partitions, N=4096 free] (channels on partitions).
  - Projections and score matmuls run in bf16 (1 cyc/row like f32r, but the
    128-col LDWEIGHTS uses fast-weight-load and overlaps the matmul stream,
    unlike f32r whose weight load serializes with the matmul).
  - Scores land transposed sT[k, q] = kT_chunk.T @ qT so the probability
    matrix is in [k-partition, q-free] layout for the PV contraction.
  - qT is pre-scaled by A_Q = 8*log2e/sqrt(C) so the score PSUM is directly
    the Schraudolph exponent: pexp bits = u8(sat(psum + B_SCH)) reinterpreted
    as fp8e4m3 gives exp(score - M_SHIFT) to ~3% (DVE path, one tensor_scalar
    with op0=add, op1=max-0 for the underflow clamp). The ACT path computes
    the exact exp via activation(Exp, scale=ln2/8, bias=-M_SHIFT) into fp8.
    Splitting the 16.7M exps between both engines keeps softmax off the
    critical path. M_SHIFT=4 centers exp(s-4) in e4m3 range (max observed
    score ~8.3, fp8 overflow at 10.05).
  - pexp tiles are [128, 2, 512] fp8 pairs; PV uses fp8 DoubleRow matmuls
    (0.5 cyc/row): one matmul per k-chunk pair with v pairs [128, 2, 128].
  - The softmax denominator Z accumulates via all-ones DoubleRow matmuls,
    grouped per q-block (16 back-to-back MMs share one LDWEIGHTS) while the
    next block's scores start.
  - The out-projection is transposed: stationary wo, moving (oT/Z) -> output
    in [C, q] layout, so the residual add uses xT directly (x_all and its
    2MB DMA are gone) and the epilogue is one scalar_tensor_tensor:
    out = (pop + bo2) + xT, with bo2 = bo + bv@wo folded host-side.
    Output DMA writes a transposed [C, N] dram tensor; host transposes back.
"""

import sys

for _p in ("/opt/trn_rl_repo",):
    if _p not in sys.path:
        sys.path.insert(0, _p)

import numpy as np

import concourse.bass as bass
import concourse.tile as tile
from concourse import bacc, mybir
from concourse.bass_utils import run_bass_kernel_spmd
from concourse.tile import add_dep_helper

B, H, W, C = 8, 64, 64, 128
N = H * W  # 4096 positions per image
GROUPS = 32
GSIZE = C // GROUPS  # 4
EPS = 1e-6
NCORES = 8
P = 128
NT = N // P  # 32 k-chunks
QB = 512  # q-block width
NQB = N // QB  # 8
NPAIR = NT // 2  # 16 k-chunk pairs per q-block
SCALE = C ** -0.5
LOG2E = 1.4426950408889634
M_SHIFT = 4.0  # softmax shift: pexp = exp(s - M_SHIFT)
A_Q = 8.0 * LOG2E * SCALE  # baked into qT so score psum = schraudolph exponent
B_SCH = 8.0 * (7.0 - LOG2E * M_SHIFT) + 0.5  # +0.5 compensates trunc-on-convert
ACT_SCALE = 1.0 / (8.0 * LOG2E)  # un-bake A_Q: exp(psum*ACT_SCALE - M_SHIFT)

F32 = mybir.dt.float32
BF16 = mybir.dt.bfloat16
F8 = mybir.dt.float8e4
U8 = mybir.dt.uint8
DR = mybir.MatmulPerfMode.DoubleRow


def build_nc():
    nc = bacc.Bacc("TRN2", target_bir_lowering=False, debug=False)

    xt_d = nc.dram_tensor("xt", [C, N], F32, kind="ExternalInput")
    wq_d = nc.dram_tensor("wq", [C, C], BF16, kind="ExternalInput")
    wk_d = nc.dram_tensor("wk", [C, C], BF16, kind="ExternalInput")
    wv_d = nc.dram_tensor("wv", [C, C], BF16, kind="ExternalInput")
    wo_d = nc.dram_tensor("wo", [C, C], BF16, kind="ExternalInput")
    bqs_d = nc.dram_tensor("bqs", [C], F32, kind="ExternalInput")  # bq * A_Q
    bk_d = nc.dram_tensor("bk", [C], F32, kind="ExternalInput")
    bo2_d = nc.dram_tensor("bo2", [C], F32, kind="ExternalInput")  # bo + bv@wo
    gns_d = nc.dram_tensor("gn_scale", [C], F32, kind="ExternalInput")
    gnb_d = nc.dram_tensor("gn_bias", [C], F32, kind="ExternalInput")
    ident_d = nc.dram_tensor("ident", [P, P], F32, kind="ExternalInput")
    gmask_d = nc.dram_tensor("gmask", [P, P], F32, kind="ExternalInput")
    out_d = nc.dram_tensor("outT", [C, N], F32, kind="ExternalOutput")

    def col(ap_1d):
        # [C] dram -> [C, 1] partition-column view
        return ap_1d.unsqueeze(1)

    with tile.TileContext(nc) as tc:
        with (
            tc.tile_pool(name="persist", bufs=1) as data,
            tc.tile_pool(name="small", bufs=1) as small,
            tc.tile_pool(name="pexp", bufs=NPAIR + 3) as pexppool,
            tc.tile_pool(name="epi", bufs=3) as epipool,
        ):
            # ---- persistent SBUF tiles ----
            xT = data.tile([P, N], F32)
            hT = data.tile([P, N], BF16)
            qTs = data.tile([P, N], BF16)  # q, pre-scaled by A_Q
            kT = data.tile([P, N], BF16)
            v_all = data.tile([P, NT, C], F8)

            wq_s = small.tile([C, C], BF16)
            wk_s = small.tile([C, C], BF16)
            wv_s = small.tile([C, C], BF16)
            wo_s = small.tile([C, C], BF16)
            ident_s = small.tile([P, P], F32)
            gmask_s = small.tile([P, P], F32)
            ones2 = small.tile([P, 2, C], F8)
            bqs_s = small.tile([C, 1], F32)
            bk_s = small.tile([C, 1], F32)
            bo2_s = small.tile([C, 1], F32)
            gns_s = small.tile([C, 1], F32)
            gnb_s = small.tile([C, 1], F32)
            eps_s = small.tile([C, 1], F32)
            negm_s = small.tile([C, 1], F32)

            # xT gates everything: split across both DMA queues.
            XCH = 4
            for ci in range(XCH):
                cs = slice(ci * N // XCH, (ci + 1) * N // XCH)
                eng = nc.sync if ci % 2 == 0 else nc.gpsimd
                eng.dma_start(xT[:, cs], xt_d[:, cs])
            # GN-chain constants on the sync queue, weights on gpsimd.
            nc.sync.dma_start(ident_s[:], ident_d[:])
            nc.sync.dma_start(gmask_s[:], gmask_d[:])
            nc.sync.dma_start(gns_s[:], col(gns_d[:]))
            nc.sync.dma_start(gnb_s[:], col(gnb_d[:]))
            nc.sync.dma_start(bqs_s[:], col(bqs_d[:]))
            nc.sync.dma_start(bk_s[:], col(bk_d[:]))
            nc.sync.dma_start(bo2_s[:], col(bo2_d[:]))
            nc.gpsimd.dma_start(wq_s[:], wq_d[:])
            nc.gpsimd.dma_start(wk_s[:], wk_d[:])
            nc.gpsimd.dma_start(wv_s[:], wv_d[:])
            nc.gpsimd.dma_start(wo_s[:], wo_d[:])
            nc.gpsimd.memset(ones2[:], 1.0)
            nc.vector.memset(eps_s[:], EPS)
            nc.vector.memset(negm_s[:], -M_SHIFT)

            # ---- phase 1+2: group norm stats straight off the xT DMA ----
            stats = small.tile([P, 16, nc.vector.BN_STATS_DIM], F32)
            with tc.tile_pool(name="tp", bufs=3, space="PSUM") as tpsum:
                for j in range(16):
                    nc.vector.bn_stats(
                        out=stats[:, j, :], in_=xT[:, j * 256 : (j + 1) * 256]
                    )
                    # keep the PE's HAM activity monitor busy through the
                    # DVE-bound stats/GN window so the attention matmuls
                    # start at full clock (idle >3.4us re-throttles).
                    pt = tpsum.tile([P, P], F32, tag="tp")
                    nc.tensor.transpose(
                        pt[0:6, :], stats[:, j, :], ident_s[:]
                    )
                mv = small.tile([P, nc.vector.BN_AGGR_DIM], F32)
                nc.vector.bn_aggr(out=mv[:], in_=stats[:])
                # per-channel [mean, E[x^2]] -> group-averaged via mask matmul
                st2 = small.tile([P, 2], F32)
                nc.vector.tensor_copy(st2[:, 0:1], mv[:, 0:1])
                msq = small.tile([P, 1], F32)
                nc.vector.tensor_mul(msq[:], mv[:, 0:1], mv[:, 0:1])
                nc.vector.tensor_add(st2[:, 1:2], mv[:, 1:2], msq[:])
                gpsum = tpsum.tile([P, 2], F32, tag="tp")
                nc.tensor.matmul(gpsum[:], gmask_s[:], st2[:])
                gstat = small.tile([P, 2], F32)
                nc.vector.tensor_copy(gstat[:], gpsum[:])

                # var_g = E_g[x^2] - mean_g^2 ; rstd = 1/sqrt(var_g + eps)
                varg = small.tile([P, 1], F32)
                nc.vector.tensor_mul(varg[:], gstat[:, 0:1], gstat[:, 0:1])
                nc.vector.tensor_tensor(
                    varg[:], gstat[:, 1:2], varg[:], mybir.AluOpType.subtract
                )
                nc.scalar.activation(
                    out=varg[:],
                    in_=varg[:],
                    func=mybir.ActivationFunctionType.Sqrt,
                    bias=eps_s[:],
                    scale=1.0,
                )
                rstd = small.tile([P, 1], F32)
                nc.vector.reciprocal(rstd[:], varg[:])
                # h = x * A + Bc with A = rstd*scale, Bc = bias - mean*A
                A_s = small.tile([P, 1], F32)
                nc.vector.tensor_mul(A_s[:], rstd[:], gns_s[:])
                mA = small.tile([P, 1], F32)
                nc.vector.tensor_mul(mA[:], gstat[:, 0:1], A_s[:])
                Bc_s = small.tile([P, 1], F32)
                nc.vector.tensor_tensor(
                    Bc_s[:], gnb_s[:], mA[:], mybir.AluOpType.subtract
                )
                # hT (bf16) in 8 chunks; alternate ACT and DVE.  A dummy PE
                # transpose paced behind each chunk keeps the HAM activity
                # window busy through this PE-idle stretch (else the array
                # re-throttles to half clock right as projections start).
                for j in range(8):
                    sl = slice(j * 512, (j + 1) * 512)
                    if j % 2 == 0:
                        hi = nc.scalar.activation(
                            out=hT[:, sl],
                            in_=xT[:, sl],
                            func=mybir.ActivationFunctionType.Identity,
                            scale=A_s[:],
                            bias=Bc_s[:],
                        )
                    else:
                        # gpsimd is SBUF-only but this op is SBUF->SBUF
                        eng = nc.gpsimd if j % 4 == 1 else nc.vector
                        hi = eng.tensor_scalar(
                            out=hT[:, sl],
                            in0=xT[:, sl],
                            scalar1=A_s[:],
                            scalar2=Bc_s[:],
                            op0=mybir.AluOpType.mult,
                            op1=mybir.AluOpType.add,
                        )
                    pt = tpsum.tile([P, P], F32, tag="tp")
                    ti = nc.tensor.transpose(pt[0:6, :], stats[:, j, :], ident_s[:])
                    add_dep_helper(ti.ins, hi.ins, sync=False, reason="ham pace")

            # ---- phase 3: projections qTs/kT [C,N] bf16, v [pos,C] fp8 ----
            with (
                tc.tile_pool(name="pq", bufs=3, space="PSUM") as pqpool,
                tc.tile_pool(name="pv", bufs=3, space="PSUM") as pvpool,
            ):
                def emit_q(j):
                    sl = slice(j * 512, (j + 1) * 512)
                    pq = pqpool.tile([P, 512], F32, tag="pq")
                    nc.tensor.matmul(pq[:], wq_s[:], hT[:, sl])
                    # qTs = A_Q*(h@wq) + A_Q*bq  (score psum = schraudolph t)
                    nc.scalar.activation(
                        out=qTs[:, sl],
                        in_=pq[:],
                        func=mybir.ActivationFunctionType.Identity,
                        scale=A_Q,
                        bias=bqs_s[:],
                    )

                for j in range(2):
                    emit_q(j)
                for j in range(8):
                    sl = slice(j * 512, (j + 1) * 512)
                    pk = pqpool.tile([P, 512], F32, tag="pq")
                    nc.tensor.matmul(pk[:], wk_s[:], hT[:, sl])
                    nc.scalar.activation(
                        out=kT[:, sl],
                        in_=pk[:],
                        func=mybir.ActivationFunctionType.Identity,
                        bias=bk_s[:],
                    )
                # v in [pos, C] fp8 (hT slice stationary); bias folded to bo2
                for i in range(NT):
                    pv = pvpool.tile([P, C], F32, tag="pv")
                    nc.tensor.matmul(pv[:], hT[:, i * P : (i + 1) * P], wv_s[:])
                    nc.scalar.copy(v_all[:, i, :], pv[:])
                for j in range(2, 8):
                    emit_q(j)

            # ---- phase 4: attention over (q-block, k-chunk-pair) steps ----
            with (
                tc.tile_pool(name="sT", bufs=4, space="PSUM") as sTpool,
                tc.tile_pool(name="oT", bufs=2, space="PSUM") as oTpool,
                tc.tile_pool(name="Zp", bufs=2, space="PSUM") as zpool,
            ):
                NSTEP = NQB * NPAIR  # 128 pair-steps
                pexp_tiles = {}
                psum_oT = {}
                psum_Z = {}
                last_score_mm = {}
                last_z_mm = {}

                def emit_scores(p):
                    # Per-half score psums (single PSUM bank each) and
                    # per-half exp: ACT takes half 0, DVE half 1, so each
                    # engine starts as soon as its own matmul lands.
                    qb, j = divmod(p, NPAIR)
                    q0 = qb * QB
                    pexp = pexppool.tile([P, 2, QB], F8, tag="pexp", name=f"pe{p}")
                    pexp_tiles[p] = pexp
                    for h in range(2):
                        kc = 2 * j + h
                        ps = sTpool.tile([P, QB], F32, tag="sT", name=f"sT{p}_{h}")
                        mi = nc.tensor.matmul(
                            ps[:],
                            kT[:, kc * P : (kc + 1) * P],
                            qTs[:, q0 : q0 + QB],
                        )
                        last_score_mm[p] = mi
                        if h == 0:
                            # ACT: exact exp(s - M) into fp8
                            nc.scalar.activation(
                                out=pexp[:, 0, :],
                                in_=ps[:],
                                func=mybir.ActivationFunctionType.Exp,
                                scale=ACT_SCALE,
                                bias=negm_s[:],
                            )
                        else:
                            # DVE: schraudolph bits = sat_u8(max(t + B, 0))
                            nc.vector.tensor_scalar(
                                out=pexp[:, 1, :].bitcast(U8),
                                in0=ps[:],
                                scalar1=B_SCH,
                                scalar2=0.0,
                                op0=mybir.AluOpType.add,
                                op1=mybir.AluOpType.max,
                            )

                def emit_pv(p):
                    qb, j = divmod(p, NPAIR)
                    if j == 0:
                        psum_oT[qb] = oTpool.tile(
                            [P, QB], F32, tag="oT", name=f"oT{qb}"
                        )
                    nc.tensor.matmul(
                        psum_oT[qb][:],
                        v_all[:, 2 * j : 2 * j + 2, :],
                        pexp_tiles[p][:],
                        start=(j == 0),
                        stop=(j == NPAIR - 1),
                        perf_mode=DR,
                    )

                def emit_z_group(qb):
                    # 16 back-to-back DoubleRow matmuls against the all-ones
                    # stationary; ordered after the lookahead scores so the
                    # burst fills the exp-latency window at block start.
                    psum_Z[qb] = zpool.tile([P, QB], F32, tag="Z", name=f"Z{qb}")
                    after = last_score_mm.get(qb * NPAIR + NPAIR + 1)
                    for j in range(NPAIR):
                        zi = nc.tensor.matmul(
                            psum_Z[qb][:],
                            ones2[:],
                            pexp_tiles[qb * NPAIR + j][:],
                            start=(j == 0),
                            stop=(j == NPAIR - 1),
                            perf_mode=DR,
                        )
                        if j == 0 and after is not None:
                            add_dep_helper(
                                zi.ins, after.ins, sync=False, reason="z after la"
                            )
                    last_z_mm[qb] = zi
                    for j in range(NPAIR):
                        del pexp_tiles[qb * NPAIR + j]

                def emit_epilogue(qb):
                    qsl = slice(qb * QB, (qb + 1) * QB)
                    poT, pZ = psum_oT.pop(qb), psum_Z.pop(qb)
                    # 1/Z = exp(-ln Z) on ACT (exp and ln share a table set;
                    # DVE reciprocal measures ~3us per 512-elem tile)
                    lnZ = epipool.tile([P, QB], F32, tag="rZ", name=f"lnZ{qb}")
                    nc.scalar.activation(
                        out=lnZ[:],
                        in_=pZ[:],
                        func=mybir.ActivationFunctionType.Ln,
                    )
                    rZ = epipool.tile([P, QB], F32, tag="rZ2", name=f"rZ{qb}")
                    nc.scalar.activation(
                        out=rZ[:],
                        in_=lnZ[:],
                        func=mybir.ActivationFunctionType.Exp,
                        scale=-1.0,
                    )
                    oTn = epipool.tile([P, QB], BF16, tag="oTn", name=f"oTn{qb}")
                    nc.vector.tensor_mul(oTn[:], poT[:], rZ[:])
                    pop = zpool.tile([P, QB], F32, tag="Z", name=f"pop{qb}")
                    nc.tensor.matmul(pop[:], wo_s[:], oTn[:])
                    outsb = epipool.tile([P, QB], F32, tag="ob", name=f"ob{qb}")
                    # out = (pop + bo2) + xT   (residual + folded biases)
                    nc.vector.scalar_tensor_tensor(
                        out=outsb[:],
                        in0=pop[:],
                        scalar=bo2_s[:],
                        in1=xT[:, qsl],
                        op0=mybir.AluOpType.add,
                        op1=mybir.AluOpType.add,
                    )
                    nc.sync.dma_start(out_d[:, qsl], outsb[:])

                LA = 2  # pair-steps of score/exp lookahead ahead of PV
                for p in range(LA):
                    emit_scores(p)
                for p in range(NSTEP):
                    qb, j = divmod(p, NPAIR)
                    emit_pv(p)
                    if p + LA < NSTEP:
                        emit_scores(p + LA)
                    if j == NPAIR - 1:
                        emit_z_group(qb)
                        emit_epilogue(qb)

    nc.compile()
    return nc


_NC_CACHE = {}


def _get_nc():
    if "nc" not in _NC_CACHE:
        _NC_CACHE["nc"] = build_nc()
    return _NC_CACHE["nc"]


def make_in_maps(**inputs):
    bf16 = mybir.dt.np(BF16)
    x = np.ascontiguousarray(np.asarray(inputs["x"], dtype=np.float32))
    ident = np.eye(P, dtype=np.float32)
    gmask = (
        np.kron(np.eye(GROUPS, dtype=np.float32), np.ones((GSIZE, GSIZE), np.float32))
        / GSIZE
    )
    wo64 = np.asarray(inputs["wo"], np.float64)
    bo2 = (
        np.asarray(inputs["bo"], np.float64)
        + np.asarray(inputs["bv"], np.float64) @ wo64
    ).astype(np.float32)
    bqs = (np.asarray(inputs["bq"], np.float64) * A_Q).astype(np.float32)
    shared = {
        "wq": np.asarray(inputs["wq"], np.float32).astype(bf16),
        "wk": np.asarray(inputs["wk"], np.float32).astype(bf16),
        "wv": np.asarray(inputs["wv"], np.float32).astype(bf16),
        "wo": np.asarray(inputs["wo"], np.float32).astype(bf16),
        "bqs": bqs,
        "bk": np.asarray(inputs["bk"], np.float32),
        "bo2": bo2,
        "gn_scale": np.asarray(inputs["gn_scale"], np.float32),
        "gn_bias": np.asarray(inputs["gn_bias"], np.float32),
        "ident": ident,
        "gmask": gmask,
    }
    return [
        {
            "xt": np.ascontiguousarray(x[b].reshape(N, C).T),
            **shared,
        }
        for b in range(B)
    ]


def kernel(**inputs):
    nc = _get_nc()
    in_maps = make_in_maps(**inputs)
    res = run_bass_kernel_spmd(nc, in_maps, core_ids=list(range(NCORES)))
    out = np.stack(
        [np.asarray(res.results[b]["outT"]).T for b in range(B)], axis=0
    )
    return out.reshape(B, H, W, C).astype(np.float32)


if __name__ == "__main__":
    rng = np.random.default_rng(0)
    ins = {
        "x": rng.standard_normal((B, H, W, C), dtype=np.float32),
        "gn_scale": np.ones(C, np.float32),
        "gn_bias": np.zeros(C, np.float32),
    }
    for w in ("wq", "wk", "wv", "wo"):
        ins[w] = rng.standard_normal((C, C), dtype=np.float32) * SCALE
    for b in ("bq", "bk", "bv", "bo"):
        ins[b] = np.zeros(C, np.float32)
    o = kernel(**ins)
    print("out", o.shape, o.dtype, float(np.abs(o).max()))


# revision 17
# speedup vs baseline: 1.6463x; 1.0584x over previous
"""AttnBlock (GroupNorm + single-head self-attention + residual) on 8 TRN2 cores.

Sharding: data-parallel over batch B=8 -> one [64,64,128] image per core.

Per-core kernel design (v2, fp8/bf16):
  - xT/hT/qT/kT are [C=128 partitions, N=4096 free] (channels on partitions).
  - Projections and score matmuls run in bf16 (1 cyc/row like f32r, but the
    128-col LDWEIGHTS uses fast-weight-load and overlaps the matmul stream,
    unlike f32r whose weight load serializes with the matmul).
  - Scores land transposed sT[k, q] = kT_chunk.T @ qT so the probability
    matrix is in [k-partition, q-free] layout for the PV contraction.
  - qT is pre-scaled by A_Q = 8*log2e/sqrt(C) so the score PSUM is directly
    the Schraudolph exponent: pexp bits = u8(sat(psum + B_SCH)) reinterpreted
    as fp8e4m3 gives exp(score - M_SHIFT) to ~3% (DVE path, one tensor_scalar
    with op0=add, op1=max-0 for the underflow clamp). The ACT path computes
    the exact exp via activation(Exp, scale=ln2/8, bias=-M_SHIFT) into fp8.
    Splitting the 16.7M exps between both engines keeps softmax off the
    critical path. M_SHIFT=4 centers exp(s-4) in e4m3 range (max observed
    score ~8.3, fp8 overflow at 10.05).
  - pexp tiles are [128, 2, 512] fp8 pairs; PV uses fp8 DoubleRow matmuls
    (0.5 cyc/row): one matmul per k-chunk pair with v pairs [128, 2, 128].
  - The softmax denominator Z accumulates via all-ones DoubleRow matmuls,
    grouped per q-block (16 back-to-back MMs share one LDWEIGHTS) while the
    next block's scores start.
  - The out-projection is transposed: stationary wo, moving (oT/Z) -> output
    in [C, q] layout, so the residual add uses xT directly (x_all and its
    2MB DMA are gone) and the epilogue is one scalar_tensor_tensor:
    out = (pop + bo2) + xT, with bo2 = bo + bv@wo folded host-side.
    Output DMA writes a transposed [C, N] dram tensor; host transposes back.
"""

import sys

for _p in ("/opt/trn_rl_repo",):
    if _p not in sys.path:
        sys.path.insert(0, _p)

import numpy as np

import concourse.bass as bass
import concourse.tile as tile
from concourse import bacc, mybir
from concourse.bass_utils import run_bass_kernel_spmd
from concourse.tile import add_dep_helper

B, H, W, C = 8, 64, 64, 128
N = H * W  # 4096 positions per image
GROUPS = 32
GSIZE = C // GROUPS  # 4
EPS = 1e-6
NCORES = 8
P = 128
NT = N // P  # 32 k-chunks
QB = 512  # q-block width
NQB = N // QB  # 8
NPAIR = NT // 2  # 16 k-chunk pairs per q-block
SCALE = C ** -0.5
LOG2E = 1.4426950408889634
M_SHIFT = 4.0  # softmax shift: pexp = exp(s - M_SHIFT)
A_Q = 8.0 * LOG2E * SCALE  # baked into qT so score psum = schraudolph exponent
B_SCH = 8.0 * (7.0 - LOG2E * M_SHIFT) + 0.5  # +0.5 compensates trunc-on-convert
ACT_SCALE = 1.0 / (8.0 * LOG2E)  # un-bake A_Q: exp(psum*ACT_SCALE - M_SHIFT)
LN2 = 0.6931471805599453
# 1/Z ~= exp(-ln2*(bits(Z)*2^-23 - 127.0450466)): schraudolph-log feeding the
# exp table (stays in the exp function set; ACT Ln would thrash table loads
# and DVE reciprocal measures ~3us per 512-elem tile). Max rel err ~3%,
# affecting only the attention path (~13% of output norm).
RZ_SCALE = -LN2 / (1 << 23)
RZ_BIAS = LN2 * (127.0 - 0.0450466)

F32 = mybir.dt.float32
BF16 = mybir.dt.bfloat16
F8 = mybir.dt.float8e4
U8 = mybir.dt.uint8
DR = mybir.MatmulPerfMode.DoubleRow


def build_nc():
    nc = bacc.Bacc("TRN2", target_bir_lowering=False, debug=False)

    xt_d = nc.dram_tensor("xt", [C, N], F32, kind="ExternalInput")
    wq_d = nc.dram_tensor("wq", [C, C], BF16, kind="ExternalInput")
    wk_d = nc.dram_tensor("wk", [C, C], BF16, kind="ExternalInput")
    wv_d = nc.dram_tensor("wv", [C, C], BF16, kind="ExternalInput")
    wo_d = nc.dram_tensor("wo", [C, C], BF16, kind="ExternalInput")
    bqs_d = nc.dram_tensor("bqs", [C], F32, kind="ExternalInput")  # bq * A_Q
    bk_d = nc.dram_tensor("bk", [C], F32, kind="ExternalInput")
    bo2_d = nc.dram_tensor("bo2", [C], F32, kind="ExternalInput")  # bo + bv@wo
    gns_d = nc.dram_tensor("gn_scale", [C], F32, kind="ExternalInput")
    gnb_d = nc.dram_tensor("gn_bias", [C], F32, kind="ExternalInput")
    ident_d = nc.dram_tensor("ident", [P, P], F32, kind="ExternalInput")
    gmask_d = nc.dram_tensor("gmask", [P, P], F32, kind="ExternalInput")
    out_d = nc.dram_tensor("outT", [C, N], F32, kind="ExternalOutput")

    def col(ap_1d):
        # [C] dram -> [C, 1] partition-column view
        return ap_1d.unsqueeze(1)

    with tile.TileContext(nc) as tc:
        with (
            tc.tile_pool(name="persist", bufs=1) as data,
            tc.tile_pool(name="small", bufs=1) as small,
            tc.tile_pool(name="pexp", bufs=NPAIR + 3) as pexppool,
            tc.tile_pool(name="epi", bufs=3) as epipool,
        ):
            # ---- persistent SBUF tiles ----
            xT = data.tile([P, N], F32)
            hT = data.tile([P, N], BF16)
            qTs = data.tile([P, N], BF16)  # q, pre-scaled by A_Q
            kT = data.tile([P, N], BF16)
            v_all = data.tile([P, NT, C], F8)

            wq_s = small.tile([C, C], BF16)
            wk_s = small.tile([C, C], BF16)
            wv_s = small.tile([C, C], BF16)
            wo_s = small.tile([C, C], BF16)
            ident_s = small.tile([P, P], F32)
            gmask_s = small.tile([P, P], F32)
            ones2 = small.tile([P, 2, C], F8)
            bqs_s = small.tile([C, 1], F32)
            bk_s = small.tile([C, 1], F32)
            bo2_s = small.tile([C, 1], F32)
            gns_s = small.tile([C, 1], F32)
            gnb_s = small.tile([C, 1], F32)
            eps_s = small.tile([C, 1], F32)
            negm_s = small.tile([C, 1], F32)
            rzb_s = small.tile([C, 1], F32)

            # xT gates everything: split across both DMA queues.
            XCH = 4
            for ci in range(XCH):
                cs = slice(ci * N // XCH, (ci + 1) * N // XCH)
                eng = nc.sync if ci % 2 == 0 else nc.gpsimd
                eng.dma_start(xT[:, cs], xt_d[:, cs])
            # GN-chain constants on the sync queue, weights on gpsimd.
            nc.sync.dma_start(ident_s[:], ident_d[:])
            nc.sync.dma_start(gmask_s[:], gmask_d[:])
            nc.sync.dma_start(gns_s[:], col(gns_d[:]))
            nc.sync.dma_start(gnb_s[:], col(gnb_d[:]))
            nc.sync.dma_start(bqs_s[:], col(bqs_d[:]))
            nc.sync.dma_start(bk_s[:], col(bk_d[:]))
            nc.sync.dma_start(bo2_s[:], col(bo2_d[:]))
            nc.gpsimd.dma_start(wq_s[:], wq_d[:])
            nc.gpsimd.dma_start(wk_s[:], wk_d[:])
            nc.gpsimd.dma_start(wv_s[:], wv_d[:])
            nc.gpsimd.dma_start(wo_s[:], wo_d[:])
            nc.gpsimd.memset(ones2[:], 1.0)
            nc.vector.memset(eps_s[:], EPS)
            nc.vector.memset(negm_s[:], -M_SHIFT)
            nc.vector.memset(rzb_s[:], RZ_BIAS)

            # ---- phase 1+2: group norm stats straight off the xT DMA ----
            stats = small.tile([P, 16, nc.vector.BN_STATS_DIM], F32)
            with tc.tile_pool(name="tp", bufs=3, space="PSUM") as tpsum:
                for j in range(16):
                    nc.vector.bn_stats(
                        out=stats[:, j, :], in_=xT[:, j * 256 : (j + 1) * 256]
                    )
                    # keep the PE's HAM activity monitor busy through the
                    # DVE-bound stats/GN window so the attention matmuls
                    # start at full clock (idle >3.4us re-throttles).
                    pt = tpsum.tile([P, P], F32, tag="tp")
                    nc.tensor.transpose(
                        pt[0:6, :], stats[:, j, :], ident_s[:]
                    )
                mv = small.tile([P, nc.vector.BN_AGGR_DIM], F32)
                nc.vector.bn_aggr(out=mv[:], in_=stats[:])
                # per-channel [mean, E[x^2]] -> group-averaged via mask matmul
                st2 = small.tile([P, 2], F32)
                nc.vector.tensor_copy(st2[:, 0:1], mv[:, 0:1])
                msq = small.tile([P, 1], F32)
                nc.vector.tensor_mul(msq[:], mv[:, 0:1], mv[:, 0:1])
                nc.vector.tensor_add(st2[:, 1:2], mv[:, 1:2], msq[:])
                gpsum = tpsum.tile([P, 2], F32, tag="tp")
                nc.tensor.matmul(gpsum[:], gmask_s[:], st2[:])
                gstat = small.tile([P, 2], F32)
                nc.vector.tensor_copy(gstat[:], gpsum[:])

                # var_g = E_g[x^2] - mean_g^2 ; rstd = 1/sqrt(var_g + eps)
                varg = small.tile([P, 1], F32)
                nc.vector.tensor_mul(varg[:], gstat[:, 0:1], gstat[:, 0:1])
                nc.vector.tensor_tensor(
                    varg[:], gstat[:, 1:2], varg[:], mybir.AluOpType.subtract
                )
                nc.scalar.activation(
                    out=varg[:],
                    in_=varg[:],
                    func=mybir.ActivationFunctionType.Sqrt,
                    bias=eps_s[:],
                    scale=1.0,
                )
                rstd = small.tile([P, 1], F32)
                nc.vector.reciprocal(rstd[:], varg[:])
                # h = x * A + Bc with A = rstd*scale, Bc = bias - mean*A
                A_s = small.tile([P, 1], F32)
                nc.vector.tensor_mul(A_s[:], rstd[:], gns_s[:])
                mA = small.tile([P, 1], F32)
                nc.vector.tensor_mul(mA[:], gstat[:, 0:1], A_s[:])
                Bc_s = small.tile([P, 1], F32)
                nc.vector.tensor_tensor(
                    Bc_s[:], gnb_s[:], mA[:], mybir.AluOpType.subtract
                )
                # hT (bf16) in 8 chunks; alternate ACT and DVE.  A dummy PE
                # transpose paced behind each chunk keeps the HAM activity
                # window busy through this PE-idle stretch (else the array
                # re-throttles to half clock right as projections start).
                for j in range(8):
                    sl = slice(j * 512, (j + 1) * 512)
                    if j % 2 == 0:
                        hi = nc.scalar.activation(
                            out=hT[:, sl],
                            in_=xT[:, sl],
                            func=mybir.ActivationFunctionType.Identity,
                            scale=A_s[:],
                            bias=Bc_s[:],
                        )
                    else:
                        # gpsimd is SBUF-only but this op is SBUF->SBUF
                        eng = nc.gpsimd if j % 4 == 1 else nc.vector
                        hi = eng.tensor_scalar(
                            out=hT[:, sl],
                            in0=xT[:, sl],
                            scalar1=A_s[:],
                            scalar2=Bc_s[:],
                            op0=mybir.AluOpType.mult,
                            op1=mybir.AluOpType.add,
                        )
                    pt = tpsum.tile([P, P], F32, tag="tp")
                    ti = nc.tensor.transpose(pt[0:6, :], stats[:, j, :], ident_s[:])
                    add_dep_helper(ti.ins, hi.ins, sync=False, reason="ham pace")

            # ---- phase 3: projections qTs/kT [C,N] bf16, v [pos,C] fp8 ----
            with (
                tc.tile_pool(name="pq", bufs=3, space="PSUM") as pqpool,
                tc.tile_pool(name="pv", bufs=3, space="PSUM") as pvpool,
            ):
                def emit_q(j):
                    sl = slice(j * 512, (j + 1) * 512)
                    pq = pqpool.tile([P, 512], F32, tag="pq")
                    nc.tensor.matmul(pq[:], wq_s[:], hT[:, sl])
                    # qTs = A_Q*(h@wq) + A_Q*bq  (score psum = schraudolph t)
                    nc.scalar.activation(
                        out=qTs[:, sl],
                        in_=pq[:],
                        func=mybir.ActivationFunctionType.Identity,
                        scale=A_Q,
                        bias=bqs_s[:],
                    )

                for j in range(2):
                    emit_q(j)
                for j in range(8):
                    sl = slice(j * 512, (j + 1) * 512)
                    pk = pqpool.tile([P, 512], F32, tag="pq")
                    nc.tensor.matmul(pk[:], wk_s[:], hT[:, sl])
                    nc.scalar.activation(
                        out=kT[:, sl],
                        in_=pk[:],
                        func=mybir.ActivationFunctionType.Identity,
                        bias=bk_s[:],
                    )
                # v in [pos, C] fp8 (hT slice stationary); bias folded to bo2
                for i in range(NT):
                    pv = pvpool.tile([P, C], F32, tag="pv")
                    nc.tensor.matmul(pv[:], hT[:, i * P : (i + 1) * P], wv_s[:])
                    nc.scalar.copy(v_all[:, i, :], pv[:])
                for j in range(2, 8):
                    emit_q(j)

            # ---- phase 4: attention over (q-block, k-chunk-pair) steps ----
            with (
                tc.tile_pool(name="sT", bufs=4, space="PSUM") as sTpool,
                tc.tile_pool(name="oT", bufs=2, space="PSUM") as oTpool,
                tc.tile_pool(name="Zp", bufs=2, space="PSUM") as zpool,
            ):
                NSTEP = NQB * NPAIR  # 128 pair-steps
                pexp_tiles = {}
                psum_oT = {}
                psum_Z = {}
                last_score_mm = {}
                last_z_mm = {}

                def emit_scores(p):
                    # Per-half score psums (single PSUM bank each) and
                    # per-half exp: ACT takes half 0, DVE half 1, so each
                    # engine starts as soon as its own matmul lands.
                    qb, j = divmod(p, NPAIR)
                    q0 = qb * QB
                    pexp = pexppool.tile([P, 2, QB], F8, tag="pexp", name=f"pe{p}")
                    pexp_tiles[p] = pexp
                    for h in range(2):
                        kc = 2 * j + h
                        ps = sTpool.tile([P, QB], F32, tag="sT", name=f"sT{p}_{h}")
                        mi = nc.tensor.matmul(
                            ps[:],
                            kT[:, kc * P : (kc + 1) * P],
                            qTs[:, q0 : q0 + QB],
                        )
                        last_score_mm[p] = mi
                        if h == 0:
                            # ACT: exact exp(s - M) into fp8
                            nc.scalar.activation(
                                out=pexp[:, 0, :],
                                in_=ps[:],
                                func=mybir.ActivationFunctionType.Exp,
                                scale=ACT_SCALE,
                                bias=negm_s[:],
                            )
                        else:
                            # DVE: schraudolph bits = sat_u8(max(t + B, 0))
                            nc.vector.tensor_scalar(
                                out=pexp[:, 1, :].bitcast(U8),
                                in0=ps[:],
                                scalar1=B_SCH,
                                scalar2=0.0,
                                op0=mybir.AluOpType.add,
                                op1=mybir.AluOpType.max,
                            )

                def emit_pv(p):
                    qb, j = divmod(p, NPAIR)
                    if j == 0:
                        psum_oT[qb] = oTpool.tile(
                            [P, QB], F32, tag="oT", name=f"oT{qb}"
                        )
                    nc.tensor.matmul(
                        psum_oT[qb][:],
                        v_all[:, 2 * j : 2 * j + 2, :],
                        pexp_tiles[p][:],
                        start=(j == 0),
                        stop=(j == NPAIR - 1),
                        perf_mode=DR,
                    )

                def emit_z_group(qb):
                    # 16 back-to-back DoubleRow matmuls against the all-ones
                    # stationary; ordered after the lookahead scores so the
                    # burst fills the exp-latency window at block start.
                    psum_Z[qb] = zpool.tile([P, QB], F32, tag="Z", name=f"Z{qb}")
                    after = last_score_mm.get(qb * NPAIR + NPAIR + 1)
                    for j in range(NPAIR):
                        zi = nc.tensor.matmul(
                            psum_Z[qb][:],
                            ones2[:],
                            pexp_tiles[qb * NPAIR + j][:],
                            start=(j == 0),
                            stop=(j == NPAIR - 1),
                            perf_mode=DR,
                        )
                        if j == 0 and after is not None:
                            add_dep_helper(
                                zi.ins, after.ins, sync=False, reason="z after la"
                            )
                    last_z_mm[qb] = zi
                    for j in range(NPAIR):
                        del pexp_tiles[qb * NPAIR + j]

                def emit_epilogue(qb):
                    qsl = slice(qb * QB, (qb + 1) * QB)
                    poT, pZ = psum_oT.pop(qb), psum_Z.pop(qb)
                    rZ = epipool.tile([P, QB], F32, tag="rZ", name=f"rZ{qb}")
                    nc.scalar.activation(
                        out=rZ[:],
                        in_=pZ[:].bitcast(mybir.dt.int32),
                        func=mybir.ActivationFunctionType.Exp,
                        scale=RZ_SCALE,
                        bias=rzb_s[:],
                    )
                    oTn = epipool.tile([P, QB], BF16, tag="oTn", name=f"oTn{qb}")
                    nc.vector.tensor_mul(oTn[:], poT[:], rZ[:])
                    pop = zpool.tile([P, QB], F32, tag="Z", name=f"pop{qb}")
                    nc.tensor.matmul(pop[:], wo_s[:], oTn[:])
                    outsb = epipool.tile([P, QB], F32, tag="ob", name=f"ob{qb}")
                    # out = (pop + bo2) + xT   (residual + folded biases)
                    nc.vector.scalar_tensor_tensor(
                        out=outsb[:],
                        in0=pop[:],
                        scalar=bo2_s[:],
                        in1=xT[:, qsl],
                        op0=mybir.AluOpType.add,
                        op1=mybir.AluOpType.add,
                    )
                    nc.sync.dma_start(out_d[:, qsl], outsb[:])

                LA = 2  # pair-steps of score/exp lookahead ahead of PV
                for p in range(LA):
                    emit_scores(p)
                for p in range(NSTEP):
                    qb, j = divmod(p, NPAIR)
                    emit_pv(p)
                    if p + LA < NSTEP:
                        emit_scores(p + LA)
                    if j == NPAIR - 1:
                        emit_z_group(qb)
                        emit_epilogue(qb)

    nc.compile()
    return nc


_NC_CACHE = {}


def _get_nc():
    if "nc" not in _NC_CACHE:
        _NC_CACHE["nc"] = build_nc()
    return _NC_CACHE["nc"]


def make_in_maps(**inputs):
    bf16 = mybir.dt.np(BF16)
    x = np.ascontiguousarray(np.asarray(inputs["x"], dtype=np.float32))
    ident = np.eye(P, dtype=np.float32)
    gmask = (
        np.kron(np.eye(GROUPS, dtype=np.float32), np.ones((GSIZE, GSIZE), np.float32))
        / GSIZE
    )
    wo64 = np.asarray(inputs["wo"], np.float64)
    bo2 = (
        np.asarray(inputs["bo"], np.float64)
        + np.asarray(inputs["bv"], np.float64) @ wo64
    ).astype(np.float32)
    bqs = (np.asarray(inputs["bq"], np.float64) * A_Q).astype(np.float32)
    shared = {
        "wq": np.asarray(inputs["wq"], np.float32).astype(bf16),
        "wk": np.asarray(inputs["wk"], np.float32).astype(bf16),
        "wv": np.asarray(inputs["wv"], np.float32).astype(bf16),
        "wo": np.asarray(inputs["wo"], np.float32).astype(bf16),
        "bqs": bqs,
        "bk": np.asarray(inputs["bk"], np.float32),
        "bo2": bo2,
        "gn_scale": np.asarray(inputs["gn_scale"], np.float32),
        "gn_bias": np.asarray(inputs["gn_bias"], np.float32),
        "ident": ident,
        "gmask": gmask,
    }
    return [
        {
            "xt": np.ascontiguousarray(x[b].reshape(N, C).T),
            **shared,
        }
        for b in range(B)
    ]


def kernel(**inputs):
    nc = _get_nc()
    in_maps = make_in_maps(**inputs)
    res = run_bass_kernel_spmd(nc, in_maps, core_ids=list(range(NCORES)))
    out = np.stack(
        [np.asarray(res.results[b]["outT"]).T for b in range(B)], axis=0
    )
    return out.reshape(B, H, W, C).astype(np.float32)


if __name__ == "__main__":
    rng = np.random.default_rng(0)
    ins = {
        "x": rng.standard_normal((B, H, W, C), dtype=np.float32),
        "gn_scale": np.ones(C, np.float32),
        "gn_bias": np.zeros(C, np.float32),
    }
    for w in ("wq", "wk", "wv", "wo"):
        ins[w] = rng.standard_normal((C, C), dtype=np.float32) * SCALE
    for b in ("bq", "bk", "bv", "bo"):
        ins[b] = np.zeros(C, np.float32)
    o = kernel(**ins)
    print("out", o.shape, o.dtype, float(np.abs(o).max()))


# revision 19
# speedup vs baseline: 1.6839x; 1.0228x over previous
"""AttnBlock (GroupNorm + single-head self-attention + residual) on 8 TRN2 cores.

Sharding: data-parallel over batch B=8 -> one [64,64,128] image per core.

Per-core kernel design (v2, fp8/bf16):
  - xT/hT/qT/kT are [C=128 partitions, N=4096 free] (channels on partitions).
  - Projections and score matmuls run in bf16 (1 cyc/row like f32r, but the
    128-col LDWEIGHTS uses fast-weight-load and overlaps the matmul stream,
    unlike f32r whose weight load serializes with the matmul).
  - Scores land transposed sT[k, q] = kT_chunk.T @ qT so the probability
    matrix is in [k-partition, q-free] layout for the PV contraction.
  - qT is pre-scaled by A_Q = 8*log2e/sqrt(C) so the score PSUM is directly
    the Schraudolph exponent: pexp bits = u8(sat(psum + B_SCH)) reinterpreted
    as fp8e4m3 gives exp(score - M_SHIFT) to ~3% (DVE path, one tensor_scalar
    with op0=add, op1=max-0 for the underflow clamp). The ACT path computes
    the exact exp via activation(Exp, scale=ln2/8, bias=-M_SHIFT) into fp8.
    Splitting the 16.7M exps between both engines keeps softmax off the
    critical path. M_SHIFT=4 centers exp(s-4) in e4m3 range (max observed
    score ~8.3, fp8 overflow at 10.05).
  - pexp tiles are [128, 2, 512] fp8 pairs; PV uses fp8 DoubleRow matmuls
    (0.5 cyc/row): one matmul per k-chunk pair with v pairs [128, 2, 128].
  - The softmax denominator Z accumulates via all-ones DoubleRow matmuls,
    grouped per q-block (16 back-to-back MMs share one LDWEIGHTS) while the
    next block's scores start.
  - The out-projection is transposed: stationary wo, moving (oT/Z) -> output
    in [C, q] layout, so the residual add uses xT directly (x_all and its
    2MB DMA are gone) and the epilogue is one scalar_tensor_tensor:
    out = (pop + bo2) + xT, with bo2 = bo + bv@wo folded host-side.
    Output DMA writes a transposed [C, N] dram tensor; host transposes back.
"""

import sys

for _p in ("/opt/trn_rl_repo",):
    if _p not in sys.path:
        sys.path.insert(0, _p)

import numpy as np

import concourse.bass as bass
import concourse.tile as tile
from concourse import bacc, mybir
from concourse.bass_utils import run_bass_kernel_spmd
from concourse.tile import add_dep_helper

B, H, W, C = 8, 64, 64, 128
N = H * W  # 4096 positions per image
GROUPS = 32
GSIZE = C // GROUPS  # 4
EPS = 1e-6
NCORES = 8
P = 128
NT = N // P  # 32 k-chunks
QB = 512  # q-block width
NQB = N // QB  # 8
NPAIR = NT // 2  # 16 k-chunk pairs per q-block
SCALE = C ** -0.5
LOG2E = 1.4426950408889634
M_SHIFT = 4.0  # softmax shift: pexp = exp(s - M_SHIFT)
A_Q = 8.0 * LOG2E * SCALE  # baked into qT so score psum = schraudolph exponent
B_SCH = 8.0 * (7.0 - LOG2E * M_SHIFT) + 0.5  # +0.5 compensates trunc-on-convert
ACT_SCALE = 1.0 / (8.0 * LOG2E)  # un-bake A_Q: exp(psum*ACT_SCALE - M_SHIFT)
LN2 = 0.6931471805599453
# 1/Z ~= exp(-ln2*(bits(Z)*2^-23 - 127.0450466)): schraudolph-log feeding the
# exp table (stays in the exp function set; ACT Ln would thrash table loads
# and DVE reciprocal measures ~3us per 512-elem tile). Max rel err ~3%,
# affecting only the attention path (~13% of output norm).
RZ_SCALE = -LN2 / (1 << 23)
RZ_BIAS = LN2 * (127.0 - 0.0450466)

F32 = mybir.dt.float32
BF16 = mybir.dt.bfloat16
F8 = mybir.dt.float8e4
U8 = mybir.dt.uint8
DR = mybir.MatmulPerfMode.DoubleRow


def build_nc():
    nc = bacc.Bacc("TRN2", target_bir_lowering=False, debug=False)

    xt_d = nc.dram_tensor("xt", [C, N], F32, kind="ExternalInput")
    wq_d = nc.dram_tensor("wq", [C, C], BF16, kind="ExternalInput")
    wk_d = nc.dram_tensor("wk", [C, C], BF16, kind="ExternalInput")
    wv_d = nc.dram_tensor("wv", [C, C], BF16, kind="ExternalInput")
    wo_d = nc.dram_tensor("wo", [C, C], BF16, kind="ExternalInput")
    bqs_d = nc.dram_tensor("bqs", [C], F32, kind="ExternalInput")  # bq * A_Q
    bk_d = nc.dram_tensor("bk", [C], F32, kind="ExternalInput")
    bo2_d = nc.dram_tensor("bo2", [C], F32, kind="ExternalInput")  # bo + bv@wo
    gns_d = nc.dram_tensor("gn_scale", [C], F32, kind="ExternalInput")
    gnb_d = nc.dram_tensor("gn_bias", [C], F32, kind="ExternalInput")
    ident_d = nc.dram_tensor("ident", [P, P], F32, kind="ExternalInput")
    gmask_d = nc.dram_tensor("gmask", [P, P], F32, kind="ExternalInput")
    out_d = nc.dram_tensor("outT", [C, N], F32, kind="ExternalOutput")

    def col(ap_1d):
        # [C] dram -> [C, 1] partition-column view
        return ap_1d.unsqueeze(1)

    with tile.TileContext(nc) as tc:
        with (
            tc.tile_pool(name="persist", bufs=1) as data,
            tc.tile_pool(name="small", bufs=1) as small,
            tc.tile_pool(name="pexp", bufs=NPAIR + 3) as pexppool,
            tc.tile_pool(name="epi", bufs=3) as epipool,
        ):
            # ---- persistent SBUF tiles ----
            xT = data.tile([P, N], F32)
            hT = data.tile([P, N], BF16)
            qTs = data.tile([P, N], BF16)  # q, pre-scaled by A_Q
            kT = data.tile([P, N], BF16)
            v_all = data.tile([P, NT, C], F8)

            wq_s = small.tile([C, C], BF16)
            wk_s = small.tile([C, C], BF16)
            wv_s = small.tile([C, C], BF16)
            wo_s = small.tile([C, C], BF16)
            ident_s = small.tile([P, P], F32)
            gmask_s = small.tile([P, P], F32)
            ones2 = small.tile([P, 2, C], F8)
            bqs_s = small.tile([C, 1], F32)
            bk_s = small.tile([C, 1], F32)
            bo2_s = small.tile([C, 1], F32)
            gns_s = small.tile([C, 1], F32)
            gnb_s = small.tile([C, 1], F32)
            eps_s = small.tile([C, 1], F32)
            negm_s = small.tile([C, 1], F32)
            rzb_s = small.tile([C, 1], F32)

            # xT gates everything: split across both DMA queues in 256-col
            # chunks matching the bn_stats slices so stats start ASAP.
            XCH = 16
            for ci in range(XCH):
                cs = slice(ci * N // XCH, (ci + 1) * N // XCH)
                eng = nc.sync if ci % 2 == 0 else nc.gpsimd
                eng.dma_start(xT[:, cs], xt_d[:, cs])
            # GN-chain constants on the sync queue, weights on gpsimd.
            nc.sync.dma_start(ident_s[:], ident_d[:])
            nc.sync.dma_start(gmask_s[:], gmask_d[:])
            nc.sync.dma_start(gns_s[:], col(gns_d[:]))
            nc.sync.dma_start(gnb_s[:], col(gnb_d[:]))
            nc.sync.dma_start(bqs_s[:], col(bqs_d[:]))
            nc.sync.dma_start(bk_s[:], col(bk_d[:]))
            nc.sync.dma_start(bo2_s[:], col(bo2_d[:]))
            nc.gpsimd.dma_start(wq_s[:], wq_d[:])
            nc.gpsimd.dma_start(wk_s[:], wk_d[:])
            nc.gpsimd.dma_start(wv_s[:], wv_d[:])
            nc.gpsimd.dma_start(wo_s[:], wo_d[:])
            nc.gpsimd.memset(ones2[:], 1.0)
            nc.vector.memset(eps_s[:], EPS)
            nc.vector.memset(negm_s[:], -M_SHIFT)
            nc.vector.memset(rzb_s[:], RZ_BIAS)

            # ---- phase 1+2: group norm stats straight off the xT DMA ----
            stats = small.tile([P, 16, nc.vector.BN_STATS_DIM], F32)
            with tc.tile_pool(name="tp", bufs=3, space="PSUM") as tpsum:
                for j in range(16):
                    nc.vector.bn_stats(
                        out=stats[:, j, :], in_=xT[:, j * 256 : (j + 1) * 256]
                    )
                    # keep the PE's HAM activity monitor busy through the
                    # DVE-bound stats/GN window so the attention matmuls
                    # start at full clock (idle >3.4us re-throttles).
                    pt = tpsum.tile([P, P], F32, tag="tp")
                    nc.tensor.transpose(
                        pt[0:6, :], stats[:, j, :], ident_s[:]
                    )
                mv = small.tile([P, nc.vector.BN_AGGR_DIM], F32)
                nc.vector.bn_aggr(out=mv[:], in_=stats[:])
                # per-channel [mean, E[x^2]] -> group-averaged via mask matmul
                st2 = small.tile([P, 2], F32)
                nc.vector.tensor_copy(st2[:, 0:1], mv[:, 0:1])
                msq = small.tile([P, 1], F32)
                nc.vector.tensor_mul(msq[:], mv[:, 0:1], mv[:, 0:1])
                nc.vector.tensor_add(st2[:, 1:2], mv[:, 1:2], msq[:])
                gpsum = tpsum.tile([P, 2], F32, tag="tp")
                nc.tensor.matmul(gpsum[:], gmask_s[:], st2[:])
                gstat = small.tile([P, 2], F32)
                nc.vector.tensor_copy(gstat[:], gpsum[:])

                # var_g = E_g[x^2] - mean_g^2 ; rstd = 1/sqrt(var_g + eps)
                varg = small.tile([P, 1], F32)
                nc.vector.tensor_mul(varg[:], gstat[:, 0:1], gstat[:, 0:1])
                nc.vector.tensor_tensor(
                    varg[:], gstat[:, 1:2], varg[:], mybir.AluOpType.subtract
                )
                nc.scalar.activation(
                    out=varg[:],
                    in_=varg[:],
                    func=mybir.ActivationFunctionType.Sqrt,
                    bias=eps_s[:],
                    scale=1.0,
                )
                rstd = small.tile([P, 1], F32)
                nc.vector.reciprocal(rstd[:], varg[:])
                # h = x * A + Bc with A = rstd*scale, Bc = bias - mean*A
                A_s = small.tile([P, 1], F32)
                nc.vector.tensor_mul(A_s[:], rstd[:], gns_s[:])
                mA = small.tile([P, 1], F32)
                nc.vector.tensor_mul(mA[:], gstat[:, 0:1], A_s[:])
                Bc_s = small.tile([P, 1], F32)
                nc.vector.tensor_tensor(
                    Bc_s[:], gnb_s[:], mA[:], mybir.AluOpType.subtract
                )
                # hT (bf16) in 8 chunks; alternate ACT and DVE.  A dummy PE
                # transpose paced behind each chunk keeps the HAM activity
                # window busy through this PE-idle stretch (else the array
                # re-throttles to half clock right as projections start).
                for j in range(8):
                    sl = slice(j * 512, (j + 1) * 512)
                    if j % 2 == 0:
                        hi = nc.scalar.activation(
                            out=hT[:, sl],
                            in_=xT[:, sl],
                            func=mybir.ActivationFunctionType.Identity,
                            scale=A_s[:],
                            bias=Bc_s[:],
                        )
                    else:
                        # gpsimd is SBUF-only but this op is SBUF->SBUF
                        eng = nc.gpsimd if j % 4 == 1 else nc.vector
                        hi = eng.tensor_scalar(
                            out=hT[:, sl],
                            in0=xT[:, sl],
                            scalar1=A_s[:],
                            scalar2=Bc_s[:],
                            op0=mybir.AluOpType.mult,
                            op1=mybir.AluOpType.add,
                        )
                    pt = tpsum.tile([P, P], F32, tag="tp")
                    ti = nc.tensor.transpose(pt[0:6, :], stats[:, j, :], ident_s[:])
                    add_dep_helper(ti.ins, hi.ins, sync=False, reason="ham pace")

            # ---- phase 3: projections qTs/kT [C,N] bf16, v [pos,C] fp8 ----
            with (
                tc.tile_pool(name="pq", bufs=3, space="PSUM") as pqpool,
                tc.tile_pool(name="pv", bufs=3, space="PSUM") as pvpool,
            ):
                def emit_q(j):
                    sl = slice(j * 512, (j + 1) * 512)
                    pq = pqpool.tile([P, 512], F32, tag="pq")
                    nc.tensor.matmul(pq[:], wq_s[:], hT[:, sl])
                    # qTs = A_Q*(h@wq) + A_Q*bq  (score psum = schraudolph t)
                    nc.scalar.activation(
                        out=qTs[:, sl],
                        in_=pq[:],
                        func=mybir.ActivationFunctionType.Identity,
                        scale=A_Q,
                        bias=bqs_s[:],
                    )

                for j in range(2):
                    emit_q(j)
                for j in range(8):
                    sl = slice(j * 512, (j + 1) * 512)
                    pk = pqpool.tile([P, 512], F32, tag="pq")
                    nc.tensor.matmul(pk[:], wk_s[:], hT[:, sl])
                    nc.scalar.activation(
                        out=kT[:, sl],
                        in_=pk[:],
                        func=mybir.ActivationFunctionType.Identity,
                        bias=bk_s[:],
                    )
                # v in [pos, C] fp8 (hT slice stationary); bias folded to bo2
                for i in range(NT):
                    pv = pvpool.tile([P, C], F32, tag="pv")
                    nc.tensor.matmul(pv[:], hT[:, i * P : (i + 1) * P], wv_s[:])
                    nc.scalar.copy(v_all[:, i, :], pv[:])
                for j in range(2, 8):
                    emit_q(j)

            # ---- phase 4: attention over (q-block, k-chunk-pair) steps ----
            # PSUM budget (8 banks): 5 single-bank score tiles + 2 oT + 1
            # shared Z/pop slot.  5 score slots deepen the critical
            # recurrence (score matmul p waits on the exp that frees slot
            # p-2.5) vs 4 slots' p-2.
            with (
                tc.tile_pool(name="sT", bufs=5, space="PSUM") as sTpool,
                tc.tile_pool(name="oT", bufs=2, space="PSUM") as oTpool,
                tc.tile_pool(name="Zp", bufs=1, space="PSUM") as zpool,
            ):
                NSTEP = NQB * NPAIR  # 128 pair-steps
                pexp_tiles = {}
                psum_oT = {}
                psum_Z = {}
                last_score_mm = {}
                last_z_mm = {}

                def emit_scores(p):
                    # Per-half score psums (single PSUM bank each) and
                    # per-half exp: ACT takes half 0, DVE half 1, so each
                    # engine starts as soon as its own matmul lands.
                    qb, j = divmod(p, NPAIR)
                    q0 = qb * QB
                    pexp = pexppool.tile([P, 2, QB], F8, tag="pexp", name=f"pe{p}")
                    pexp_tiles[p] = pexp
                    for h in range(2):
                        kc = 2 * j + h
                        ps = sTpool.tile([P, QB], F32, tag="sT", name=f"sT{p}_{h}")
                        mi = nc.tensor.matmul(
                            ps[:],
                            kT[:, kc * P : (kc + 1) * P],
                            qTs[:, q0 : q0 + QB],
                        )
                        last_score_mm[p] = mi
                        if h == 0:
                            # ACT: exact exp(s - M) into fp8
                            nc.scalar.activation(
                                out=pexp[:, 0, :],
                                in_=ps[:],
                                func=mybir.ActivationFunctionType.Exp,
                                scale=ACT_SCALE,
                                bias=negm_s[:],
                            )
                        else:
                            # DVE: schraudolph bits = sat_u8(max(t + B, 0))
                            nc.vector.tensor_scalar(
                                out=pexp[:, 1, :].bitcast(U8),
                                in0=ps[:],
                                scalar1=B_SCH,
                                scalar2=0.0,
                                op0=mybir.AluOpType.add,
                                op1=mybir.AluOpType.max,
                            )

                def emit_pv(p):
                    qb, j = divmod(p, NPAIR)
                    if j == 0:
                        psum_oT[qb] = oTpool.tile(
                            [P, QB], F32, tag="oT", name=f"oT{qb}"
                        )
                    nc.tensor.matmul(
                        psum_oT[qb][:],
                        v_all[:, 2 * j : 2 * j + 2, :],
                        pexp_tiles[p][:],
                        start=(j == 0),
                        stop=(j == NPAIR - 1),
                        perf_mode=DR,
                    )

                def emit_z_group(qb):
                    # 16 back-to-back DoubleRow matmuls against the all-ones
                    # stationary; ordered after the lookahead scores so the
                    # burst fills the exp-latency window at block start.
                    psum_Z[qb] = zpool.tile([P, QB], F32, tag="Z", name=f"Z{qb}")
                    after = last_score_mm.get(qb * NPAIR + NPAIR + 1)
                    for j in range(NPAIR):
                        zi = nc.tensor.matmul(
                            psum_Z[qb][:],
                            ones2[:],
                            pexp_tiles[qb * NPAIR + j][:],
                            start=(j == 0),
                            stop=(j == NPAIR - 1),
                            perf_mode=DR,
                        )
                        if j == 0 and after is not None:
                            add_dep_helper(
                                zi.ins, after.ins, sync=False, reason="z after la"
                            )
                    last_z_mm[qb] = zi
                    for j in range(NPAIR):
                        del pexp_tiles[qb * NPAIR + j]

                def emit_epilogue(qb):
                    qsl = slice(qb * QB, (qb + 1) * QB)
                    poT, pZ = psum_oT.pop(qb), psum_Z.pop(qb)
                    rZ = epipool.tile([P, QB], F32, tag="rZ", name=f"rZ{qb}")
                    nc.scalar.activation(
                        out=rZ[:],
                        in_=pZ[:].bitcast(mybir.dt.int32),
                        func=mybir.ActivationFunctionType.Exp,
                        scale=RZ_SCALE,
                        bias=rzb_s[:],
                    )
                    oTn = epipool.tile([P, QB], BF16, tag="oTn", name=f"oTn{qb}")
                    nc.vector.tensor_mul(oTn[:], poT[:], rZ[:])
                    pop = zpool.tile([P, QB], F32, tag="Z", name=f"pop{qb}")
                    nc.tensor.matmul(pop[:], wo_s[:], oTn[:])
                    outsb = epipool.tile([P, QB], F32, tag="ob", name=f"ob{qb}")
                    # out = (pop + bo2) + xT   (residual + folded biases)
                    nc.vector.scalar_tensor_tensor(
                        out=outsb[:],
                        in0=pop[:],
                        scalar=bo2_s[:],
                        in1=xT[:, qsl],
                        op0=mybir.AluOpType.add,
                        op1=mybir.AluOpType.add,
                    )
                    nc.sync.dma_start(out_d[:, qsl], outsb[:])

                LA = 2  # pair-steps of score/exp lookahead ahead of PV
                for p in range(LA):
                    emit_scores(p)
                for p in range(NSTEP):
                    qb, j = divmod(p, NPAIR)
                    emit_pv(p)
                    if p + LA < NSTEP:
                        emit_scores(p + LA)
                    if j == NPAIR - 1:
                        emit_z_group(qb)
                        emit_epilogue(qb)

    nc.compile()
    return nc


_NC_CACHE = {}


def _get_nc():
    if "nc" not in _NC_CACHE:
        _NC_CACHE["nc"] = build_nc()
    return _NC_CACHE["nc"]


def make_in_maps(**inputs):
    bf16 = mybir.dt.np(BF16)
    x = np.ascontiguousarray(np.asarray(inputs["x"], dtype=np.float32))
    ident = np.eye(P, dtype=np.float32)
    gmask = (
        np.kron(np.eye(GROUPS, dtype=np.float32), np.ones((GSIZE, GSIZE), np.float32))
        / GSIZE
    )
    wo64 = np.asarray(inputs["wo"], np.float64)
    bo2 = (
        np.asarray(inputs["bo"], np.float64)
        + np.asarray(inputs["bv"], np.float64) @ wo64
    ).astype(np.float32)
    bqs = (np.asarray(inputs["bq"], np.float64) * A_Q).astype(np.float32)
    shared = {
        "wq": np.asarray(inputs["wq"], np.float32).astype(bf16),
        "wk": np.asarray(inputs["wk"], np.float32).astype(bf16),
        "wv": np.asarray(inputs["wv"], np.float32).astype(bf16),
        "wo": np.asarray(inputs["wo"], np.float32).astype(bf16),
        "bqs": bqs,
        "bk": np.asarray(inputs["bk"], np.float32),
        "bo2": bo2,
        "gn_scale": np.asarray(inputs["gn_scale"], np.float32),
        "gn_bias": np.asarray(inputs["gn_bias"], np.float32),
        "ident": ident,
        "gmask": gmask,
    }
    return [
        {
            "xt": np.ascontiguousarray(x[b].reshape(N, C).T),
            **shared,
        }
        for b in range(B)
    ]


def kernel(**inputs):
    nc = _get_nc()
    in_maps = make_in_maps(**inputs)
    res = run_bass_kernel_spmd(nc, in_maps, core_ids=list(range(NCORES)))
    out = np.stack(
        [np.asarray(res.results[b]["outT"]).T for b in range(B)], axis=0
    )
    return out.reshape(B, H, W, C).astype(np.float32)


if __name__ == "__main__":
    rng = np.random.default_rng(0)
    ins = {
        "x": rng.standard_normal((B, H, W, C), dtype=np.float32),
        "gn_scale": np.ones(C, np.float32),
        "gn_bias": np.zeros(C, np.float32),
    }
    for w in ("wq", "wk", "wv", "wo"):
        ins[w] = rng.standard_normal((C, C), dtype=np.float32) * SCALE
    for b in ("bq", "bk", "bv", "bo"):
        ins[b] = np.zeros(C, np.float32)
    o = kernel(**ins)
    print("out", o.shape, o.dtype, float(np.abs(o).max()))
